# revision 93
# baseline (speedup 1.0000x reference)
"""Causal self-attention (B=2, T=2048, C=1024, H=16, D=64) on 8 trn2 cores.

Sharding: core c handles batch b = c//4 and head group hg = c%4 (heads
4*hg .. 4*hg+3).  Each core computes q/k/v projections for its 4 heads,
causal-softmax attention, and a partial output projection
y_partial = O_heads @ Wo[:, heads].T.  The host sums the 4 partials per
batch and adds the bias.

Numerics (measured 1.21e-2 max-rel vs the 2e-2 gate):
  - q/k projections run in fp8e4m3 with perf_mode=DoubleRow (K=256 per
    matmul): weights pre-scaled by 16 (folded back out via the exp
    scale), activations straight-cast.
  - S = k^T q ALSO runs fp8 DoubleRow (half PE cost per column): the
    projection PSUM is re-quantized to e4m3 and repacked into the
    DoubleRow operand layout (see below).  Double quantization of q,k
    raises max-rel from ~0.9e-2 to ~1.2e-2.
  - V/output projections, P, O use bf16 operands with fp32 PSUM.

Layout:
  q/k DR operands: the projection weights' M columns are host-permuted
          to [e0:32|o0:32|e32:64|o32:64] per pair, so the projection
          PSUM rows land in DoubleRow *plane* order.  Per (pair, chunk):
          one DVE tensor_scalar copy PSUM -> qkf8 [128, 1024] f8
          (q cols 0:512, k 512:1024), then two partition-block DMAs
          fold it into qkdr [64, 2, 1024]: partition p<32 head-even,
          32:64 head-odd; ko plane 0 = dims 0:32, plane 1 = 32:64.
          S matmuls then use lhsT/rhs [32, 2, cols] with Ki=32, K=64.
  V is projected PRE-TRANSPOSED (lhsT = xc tile, rhs = wv tile) into
          [128 Tk, 256] PSUM, then one strided copy per Tk tile into
          vaug[t] [128, 260] = [Vh0|1|Vh1|1|Vh2|1|Vh3|1]; the ones
          columns (memset once) make PSUM row 64 of the O^T
          accumulation the softmax denominator
  S^T_j = kdr_j^T @ qdr_I  (fp8 DR, k-major, causally trimmed)
  P = exp(S/(sqrt(C)*256)) on ACT, batched [128,1024] per head-group;
          the 2-act diagonal split is merged into ONE strided 3D-AP act
          (over-computing bounded stale cols nothing reads); the
          boundary strip is masked on DVE
  O^T_h = sum_j vaug[j][:, 65h:65h+65].T @ P_j  -> [65, 512] PSUM
  normalize: reciprocal (DVE) -> partition_broadcast (GPSIMD) -> mul;
          head-odd half shifted to partitions 64:128 via SBUF DMA --
          processed h=1 FIRST since its chain is one DMA longer and
          gates the y units (worth ~1.5us over the 16 instances)
  y: per (Tk, 512-col) unit, 2 matmuls (K=256 over the head pairs) ->
          bf16 copy into a persistent per-chunk staging tile -> one
          128-row DMA per Tk tile

Emission / scheduling: the Tile scheduler is a readiness+priority list
scheduler; the per-engine streams it execute in order.  With S in fp8
the kernel is ACT(exp)-bound (~86us ACT vs ~79us PE busy), so the
emission keeps the exp stream fed: next-chunk projection groups and
y-projection units are interleaved into the attention stream as PE
filler, paced by an ACT-vs-PE deficit tracker (pay_debt, pop-capped so
a debt spike cannot push the next S group past one exp).  DMA rules
learned from the timeline model: every DRAM tensor is host-packed
per-partition-contiguous (strided DRAM APs cost 1-4us of descriptor
generation on the issuing sequencer); HWDGE issue is ~626ns serial per
DMA and the DMA device runs one transfer at a time, so the lead-in
keeps the pre-fold DMA count minimal and later chunks' x transfers are
emitted lazily at the previous chunk's attention start; a DMA issue
seq-WAITS on its input dependency while holding its queue, so the fold
DMAs live on the sync queue (never ahead of latency-critical exp
issues on the ACT queue); chunk-0 k-copies ride the then-idle ACT
engine and chunk-0 V t1..t3 are deferred into the attention stream so
the first S/exp is not queued behind them on the in-order PE stream.
"""
import numpy as np
import ml_dtypes

import concourse.tile as tile
import concourse.mybir as mybir
from concourse import bacc
from concourse.bass_utils import run_bass_kernel_spmd

FP = mybir.dt.float32
BF = mybir.dt.bfloat16
F8 = mybir.dt.float8e4
W8SCALE = 16.0  # fp8 weight pre-scale (folded back out via the exp scale)
NDT = 4  # DoubleRow K-tiles over the embedding dim (4 x (128x2))
B, T, C = 2, 2048, 1024
H, D = 16, 64
SCALE = 1.0 / 32.0  # 1/sqrt(C)
S_SCALE = SCALE / (W8SCALE * W8SCALE)  # q,k carry a 16x pre-scale each
N_CORES = 8
NKT = C // 128  # 8 K-tiles over the embedding dim
NTK = T // 128  # 16 Tk tiles
NI = T // 512  # 4 Tq chunks
EXP = mybir.ActivationFunctionType.Exp

_nc_cache = {}


def build_kernel(repeats=1, hmix=False):
    key = (repeats, hmix)
    if key in _nc_cache:
        return _nc_cache[key]

    nc = bacc.Bacc("TRN2", target_bir_lowering=False, debug=False)

    # all DRAM inputs are host-packed so every DMA reads per-partition
    # CONTIGUOUS bytes: strided DRAM APs cost 1-4us of descriptor
    # generation on the issuing sequencer, which stalled the lead-in
    xc_d = nc.dram_tensor("xc", [128, NI, NKT, 512], BF, kind="ExternalInput").ap()
    xq8_d = nc.dram_tensor("xq8", [128, NI, NDT, 2, 512], F8, kind="ExternalInput").ap()
    wqk8_d = nc.dram_tensor("wqk8", [128, 2, NDT, 2, 256], F8, kind="ExternalInput").ap()
    wv_d = nc.dram_tensor("wv", [128, NKT, 256], BF, kind="ExternalInput").ap()
    wo_d = nc.dram_tensor("wo", [128, 2, C], BF, kind="ExternalInput").ap()
    y_d = nc.dram_tensor("y", [T, C], BF, kind="ExternalOutput").ap()

    # mask[p, i] = 1 iff i >= p : triangular strip at the causal boundary
    mask_np = (
        np.arange(128)[None, :] >= np.arange(128)[:, None]
    ).astype(ml_dtypes.bfloat16)
    mask_d = nc.inline_tensor(mask_np, "mask_tri").ap()

    with tile.TileContext(nc) as tc:
        with (
            tc.tile_pool(name="persist", bufs=1) as pp,
            tc.tile_pool(name="ppool", bufs=20) as ppool,
            tc.tile_pool(name="spool", bufs=4) as spool,
            tc.tile_pool(name="qkf", bufs=4) as qkf_pool,
            tc.tile_pool(name="ps_s", bufs=2, space="PSUM") as ps_s,
            tc.tile_pool(name="ps_o", bufs=2, space="PSUM") as ps_o,
            tc.tile_pool(name="ps_mm", bufs=2, space="PSUM") as ps_mm,
        ):
            # ---- critical-path DMAs first.  HWDGE issue is ~626ns serial
            # per DMA, so the pre-fold lead-in keeps the DMA count minimal:
            # scalar queue [wqk8, wv, folds...], sync queue [xq8[0], xc[0],
            # mask, rest] ----
            wqk8 = pp.tile([128, 2, NDT, 2, 256], F8, tag="wqk8")
            nc.scalar.dma_start(wqk8[:, 0, :, :, :], wqk8_d[:, 0, :, :, :])
            xq8 = [
                pp.tile([128, NDT, 2, 512], F8, tag=f"xq8{c}", name=f"xq8_{c}")
                for c in range(NI)
            ]
            nc.sync.dma_start(xq8[0][:, 0:2, :, :], xq8_d[:, 0, 0:2, :, :])
            nc.scalar.dma_start(wqk8[:, 1, :, :, :], wqk8_d[:, 1, :, :, :])
            nc.sync.dma_start(xq8[0][:, 2:4, :, :], xq8_d[:, 0, 2:4, :, :])
            wv_big = pp.tile([128, NKT, 256], BF, tag="wv")
            nc.scalar.dma_start(wv_big[:, :, :], wv_d[:, :, :])
            # x for the V projection; one contiguous DMA per chunk
            xc = [
                pp.tile([128, NKT, 512], BF, tag=f"xc{c}", name=f"xc{c}")
                for c in range(NI)
            ]
            nc.sync.dma_start(xc[0][:, 0:4, :], xc_d[:, 0, 0:4, :])
            nc.sync.dma_start(xc[0][:, 4:8, :], xc_d[:, 0, 4:8, :])
            mask = pp.tile([128, 128], BF, tag="mask")
            nc.sync.dma_start(mask[:], mask_d[:])
            # x for chunks >= 1 and wo are DMA'd lazily at the start of the
            # PREVIOUS chunk's attention: the DMA device serializes
            # transfers in issue order, and the chunk-c fold DMAs must not
            # queue behind 3 chunks of x traffic (2.9us each)
            wo_big = pp.tile([128, 2, C], BF, tag="wo")

            def emit_late_dmas(c):  # called at start of attention chunk c
                if R[0] > 0:
                    return  # inputs already resident on later repeats
                if c + 1 < NI:
                    nc.sync.dma_start(
                        xq8[c + 1][:, :, :, :], xq8_d[:, c + 1, :, :, :]
                    )
                    nc.sync.dma_start(xc[c + 1][:, :, :], xc_d[:, c + 1, :, :])
                if c == 1:
                    nc.sync.dma_start(wo_big[:, :, :], wo_d[:, :, :])

            # preload the exp table while DMAs stream (saves ~1.3us later)
            warm_in = pp.tile([1, 2], FP, tag="warm_in")
            warm_out = pp.tile([1, 2], FP, tag="warm_out")
            nc.vector.memset(warm_in[:], 0.0)
            nc.scalar.activation(warm_out[:], warm_in[:], EXP, scale=SCALE)

            # ---- persistent activations ----
            # qkf8[pair][c]: [128, 1024] f8 staging (q cols 0:512, k cols
            # 512:1024), rows in DR-plane order [e0:32|o0:32|e32:64|o32:64]
            # (host-permuted W cols).  Folded by 2 DMAs into qkdr[pair][c]
            # [64, 2, 1024]: partition p<32 head-even, 32:64 head-odd; ko
            # plane 0 = dims 0:32, plane 1 = dims 32:64.  S matmuls then run
            # fp8 DoubleRow (Ki=32, K=64) at half PE cost.
            qkf8 = [[None] * NI for _ in range(2)]
            qkdr = [
                [pp.tile([64, 2, 1024], F8, tag=f"qkd{p}_{i}", name=f"qkd{p}_{i}") for i in range(NI)]
                for p in range(2)
            ]
            otstc = [
                [pp.tile([128, 512], BF, tag=f"ot{p}_{i}", name=f"ot{p}_{i}") for i in range(NI)]
                for p in range(2)
            ]
            # vaug[t] = [Vh0|1|Vh1|1|Vh2|1|Vh3|1]; ones set once, V cols
            # rewritten per repeat by the strided copy from the V psum
            vaug = [pp.tile([128, 260], BF, tag=f"va{t}", name=f"va{t}") for t in range(NTK)]
            for t in range(NTK):
                nc.vector.memset(vaug[t][:, 64:260:65], 1.0)

            R = [0]

            # ---- PE filler machinery: queues of (cost_ns, emit_fn).
            # proj fillers must all land before the next attention chunk;
            # yproj fillers may linger until the final drain ----
            fillers_proj = []
            fillers_y = []
            debt = [0.0]
            y_reserve = [0]

            def pay_debt(max_pops=3, allow_proj=True):
                # cap pops per call so a debt spike cannot push the next
                # attention group's S matmuls out by more than ~one exp.
                # allow_proj=False while the next chunk's x transfers are
                # still in flight: a popped projection matmul would wait on
                # them IN the in-order PE stream, stalling attention behind
                while max_pops > 0 and debt[0] > 0.0 and (
                    (fillers_proj and allow_proj)
                    or len(fillers_y) > y_reserve[0]
                ):
                    q = (
                        fillers_proj
                        if (fillers_proj and allow_proj)
                        else fillers_y
                    )
                    cost, fn = q.pop(0)
                    fn()
                    debt[0] -= cost
                    max_pops -= 1

            def drain_proj_fillers():
                while fillers_proj:
                    _, fn = fillers_proj.pop(0)
                    fn()
                debt[0] = 0.0

            def drain_y_fillers(on_act=False):
                while fillers_y:
                    _, fn = fillers_y.pop(0)
                    fn(on_act=on_act)

            def drain_all_fillers():
                drain_proj_fillers()
                drain_y_fillers(on_act="tail")

            # ---- emission units ----
            def emit_qk_group(c, nm, pair, copy_on_act=False):
                qk = 0 if nm == "q" else 1
                ps = ps_mm.tile([128, 512], FP, tag="mm",
                                name=f"ps{nm}{c}_{pair}_r{R[0]}")
                for dt in range(NDT):
                    nc.tensor.matmul(
                        ps[:],
                        lhsT=wqk8[:, qk, dt, :, pair * 128 : pair * 128 + 128],
                        rhs=xq8[c][:, dt, :, :],
                        start=(dt == 0),
                        stop=(dt == NDT - 1),
                        perf_mode=mybir.MatmulPerfMode.DoubleRow,
                    )
                # this copy gates the next chunk's whole attention stream;
                # schedule it ahead of other queued DVE work
                if nm == "q":
                    qkf8[pair][c] = qkf_pool.tile(
                        [128, 1024], F8, tag="qkf", name=f"qkf{pair}_{c}_r{R[0]}"
                    )
                half = slice(0, 512) if nm == "q" else slice(512, 1024)
                with tc.high_priority(60):
                    if copy_on_act:
                        nc.scalar.copy(qkf8[pair][c][:, half], ps[:])
                    else:
                        nc.vector.tensor_scalar_mul(
                            qkf8[pair][c][:, half], ps[:], 1.0
                        )
                if nm == "k":  # both halves staged -> fold into DR layout.
                    # sync queue: a DMA issue seq-WAITS on its input dep
                    # while holding its queue, so folds must not share a
                    # queue with latency-critical issues (exp is on ACT)
                    with tc.high_priority(60):
                        nc.sync.dma_start(
                            qkdr[pair][c][:, 0, :], qkf8[pair][c][0:64, :]
                        )
                        nc.sync.dma_start(
                            qkdr[pair][c][:, 1, :], qkf8[pair][c][64:128, :]
                        )

            def emit_v_group(t):
                c, ts = t // 4, t % 4
                ps = ps_mm.tile([128, 512], FP, tag="mm",
                                name=f"psv{t}_r{R[0]}")
                for kk in range(NKT):
                    nc.tensor.matmul(
                        ps[:, 0:256],
                        lhsT=xc[c][:, kk, ts * 128 : ts * 128 + 128],
                        rhs=wv_big[:, kk, :],
                        start=(kk == 0),
                        stop=(kk == NKT - 1),
                    )
                with tc.high_priority(60):
                    nc.vector.tensor_copy(
                        vaug[t][:].rearrange("p (n d) -> p n d", n=4)[:, :, 0:64],
                        ps[:, 0:256].rearrange("p (n d) -> p n d", n=4),
                    )

            def emit_proj_chunk(c):
                # k-copies ride the (idle) ACT engine at the lead-in so the
                # fold does not wait behind the q-copy on DVE; V t1..t3 are
                # deferred into the attention stream so the first S/exp is
                # not queued behind 24 V matmuls on the in-order PE stream
                on_act = R[0] == 0
                for pair in range(2):
                    emit_qk_group(c, "q", pair)
                    emit_qk_group(c, "k", pair, copy_on_act=on_act)
                emit_v_group(4 * c)
                for t in range(4 * c + 1, 4 * c + 4):
                    fillers_proj.insert(
                        t - 4 * c - 1, (870, lambda t=t: emit_v_group(t))
                    )

            def proj_chunk_fillers(c):
                for pair in range(2):
                    fillers_proj.append((430, lambda pair=pair: emit_qk_group(
                        c, "q", pair)))
                    fillers_proj.append((430, lambda pair=pair: emit_qk_group(
                        c, "k", pair)))
                for t in range(4 * c, 4 * c + 4):
                    fillers_proj.append((870, lambda t=t: emit_v_group(t)))

            yt_chunk = [
                pp.tile([128, 4, 1024], BF, tag=f"yc{c}", name=f"yc{c}")
                for c in range(NI)
            ]
            y_done = {}

            def emit_yproj_unit(t, nch, on_act=False):
                ps = ps_mm.tile([128, 512], FP, tag="mm",
                                name=f"psy{t}_{nch}_r{R[0]}")
                for pair in range(2):
                    nc.tensor.matmul(
                        ps[:],
                        lhsT=otstc[pair][t // 4][
                            :, (t % 4) * 128 : (t % 4) * 128 + 128
                        ],
                        rhs=wo_big[:, pair, nch * 512 : nch * 512 + 512],
                        start=(pair == 0),
                        stop=(pair == 1),
                    )
                c = t // 4
                dst = yt_chunk[c][:, t % 4, nch * 512 : nch * 512 + 512]
                # mid-run copies go to DVE (ACT is exp-saturated); in the
                # reserve drain the DVE queue is full of normalize work so
                # use ACT; at the tail both are idle, so alternate
                if on_act == "act" or (
                    on_act == "tail" and (2 * t + nch) % 2 == 0
                ):
                    nc.scalar.copy(dst, ps[:])
                else:
                    nc.vector.tensor_copy(dst, ps[:])
                y_done[c] = y_done.get(c, 0) + 1
                if y_done[c] in (2, 4, 6, 8):  # quarter-chunk -> DMA it
                    lo = y_done[c] // 2 - 1
                    nc.sync.dma_start(
                        y_d[c * 512 + lo * 128 : c * 512 + lo * 128 + 128, :]
                        .rearrange("(n p) d -> p n d", p=128),
                        yt_chunk[c][:, lo : lo + 1, :],
                    )
                    if y_done[c] == 8:
                        y_done[c] = 0

            def emit_attention(I, last=False, pairs=(0, 1)):
                emit_attention_body(I, last, pairs)
                if 1 in pairs:
                    for t in range(4 * I, 4 * I + 4):
                        for nch in range(2):
                            fillers_y.append(
                                (430,
                                 lambda t=t, nch=nch, **kw: emit_yproj_unit(
                                     t, nch, **kw))
                            )

            def emit_attention_body(I, last, pairs):
                jmax = 4 * I + 4
                for pair in pairs:
                    oT = [
                        ps_o.tile([65, 512], FP, tag="oT",
                                  name=f"o{I}_{pair}_{h}_r{R[0]}")
                        for h in range(2)
                    ]

                    def emit_o(g, tiles):
                        j0 = 2 * g
                        for h in range(2):
                            p_sb = tiles[h]
                            head = 2 * pair + h
                            for dj in range(2):
                                j = j0 + dj
                                z = max(0, j * 128 - I * 512)
                                nc.tensor.matmul(
                                    oT[h][:, z:512],
                                    lhsT=vaug[j][:, 65 * head : 65 * head + 65],
                                    rhs=p_sb[:, dj * 512 + z : dj * 512 + 512],
                                    start=(j == 0),
                                    stop=(j == jmax - 1),
                                )

                    prev = None
                    for g in range(jmax // 2):
                        j0 = 2 * g
                        diag = j0 >= 4 * I
                        zs = [max(0, (j0 + dj) * 128 - I * 512) for dj in range(2)]
                        cur = []
                        act_ns = 0.0
                        cols = 0
                        for h in range(2):
                            hsl = slice(32 * h, 32 * h + 32)
                            s_ps = ps_s.tile([128, 1024], FP, tag="s",
                                             name=f"s{I}_{pair}_{h}_{g}_r{R[0]}")
                            for dj in range(2):
                                j = j0 + dj
                                z = zs[dj]
                                kof = 512 + (j % 4) * 128
                                nc.tensor.matmul(
                                    s_ps[:, dj * 512 + z : dj * 512 + 512],
                                    lhsT=qkdr[pair][j // 4][
                                        hsl, :, kof : kof + 128
                                    ],
                                    rhs=qkdr[pair][I][hsl, :, z:512],
                                    start=True,
                                    stop=True,
                                    perf_mode=mybir.MatmulPerfMode.DoubleRow,
                                )
                                cols += 512 - z
                            p_sb = ppool.tile([128, 1024], BF, tag="p",
                                              name=f"p{I}_{pair}_{h}_{g}_r{R[0]}")
                            if not diag or zs[0] == 0:
                                # diag group with z0=0: one act over the whole
                                # tile; the uncomputed gap [512:512+z1] holds
                                # stale S values, bounded so exp stays finite,
                                # and nothing downstream reads it
                                nc.scalar.activation(p_sb[:], s_ps[:], EXP,
                                                     scale=S_SCALE)
                                act_ns += 1024 * 0.833 + 185
                            else:
                                # one strided act at the smaller z: the
                                # over-covered [512+z0:512+z1] region holds
                                # bounded stale S (exp stays finite) and is
                                # never read downstream
                                nc.scalar.activation(
                                    p_sb[:].rearrange(
                                        "p (d t) -> p d t", d=2
                                    )[:, :, zs[0] : 512],
                                    s_ps[:].rearrange(
                                        "p (d t) -> p d t", d=2
                                    )[:, :, zs[0] : 512],
                                    EXP,
                                    scale=S_SCALE,
                                )
                                act_ns += 2 * (512 - zs[0]) * 0.833 + 185
                            for dj in range(2):
                                j = j0 + dj
                                if j >= 4 * I:
                                    z = zs[dj]
                                    ssl = slice(dj * 512 + z, dj * 512 + z + 128)
                                    nc.vector.tensor_mul(
                                        p_sb[:, ssl], p_sb[:, ssl], mask[:]
                                    )
                            cur.append(p_sb)
                        # ACT-vs-PE deficit for this group: exp time vs the
                        # S (fp8 DR, 0.2083/col) + O (bf16, 0.4167/col) time
                        debt[0] += act_ns - cols * 0.625
                        if prev is not None:
                            emit_o(g - 1, prev)
                        pay_debt()
                        prev = cur
                    emit_o(jmax // 2 - 1, prev)
                    if last and pair == 1:
                        # reserved y units keep PE warm through the final
                        # normalize chain
                        y_reserve[0] = 0
                        drain_y_fillers(on_act="act")
                    # normalize O^T by the PSUM row-64 denominator;
                    # h=1 first: its chain is longer (partition-shift DMA)
                    for h in (1, 0):
                        recip = spool.tile([1, 512], FP, tag="recip",
                                           name=f"rc{I}_{pair}_{h}_r{R[0]}")
                        nc.vector.reciprocal(recip[:], oT[h][64:65, :])
                        bcast = spool.tile([64, 512], FP, tag="bcast",
                                           name=f"bc{I}_{pair}_{h}_r{R[0]}")
                        nc.gpsimd.partition_broadcast(bcast[:], recip[:])
                        if h == 0:
                            nc.vector.tensor_mul(
                                otstc[pair][I][0:64, :], oT[h][0:64, :], bcast[:]
                            )
                        else:
                            onrm = spool.tile([64, 512], BF, tag="onrm",
                                              name=f"on{I}_{pair}_r{R[0]}")
                            nc.vector.tensor_mul(onrm[:], oT[h][0:64, :], bcast[:])
                            # partition shift 0->64 needs a DMA
                            nc.sync.dma_start(otstc[pair][I][64:128, :], onrm[:])


            # ---- main emission.  Attention instructions carry high
            # scheduler priority (they form the serial latency chain:
            # S -> exp -> mask -> O -> normalize); projections and
            # y-projection units are normal priority, so the greedy
            # scheduler slots them into PE whenever attention work is
            # not ready ----
            for rep in range(repeats):
                R[0] = rep
                emit_proj_chunk(0)
                for c in range(NI):
                    emit_late_dmas(c)
                    if c + 1 < NI:
                        proj_chunk_fillers(c + 1)
                    else:
                        y_reserve[0] = 8
                    if c < 2:
                        # early chunks have little attention work: drain
                        # next-chunk projections eagerly so their folds land
                        # before this chunk's exp stream runs dry
                        debt[0] += 3500.0
                    emit_attention(c, last=(c == NI - 1))
                    drain_proj_fillers()
                y_reserve[0] = 0
                drain_all_fillers()

    nc.compile()
    _nc_cache[key] = nc
    return nc


def make_in_maps(x, Wq, Wk, Wv, Wo):
    x = np.asarray(x, dtype=np.float32)
    Wq = np.asarray(Wq, dtype=np.float32)
    Wk = np.asarray(Wk, dtype=np.float32)
    Wv = np.asarray(Wv, dtype=np.float32)
    Wo = np.asarray(Wo, dtype=np.float32)
    bf = ml_dtypes.bfloat16
    f8 = ml_dtypes.float8_e4m3fn

    def dr_pack(a):  # [C, m] -> [128, NDT, 2, m] with k = 256*dt + ki + 128*ko
        return np.ascontiguousarray(
            a.reshape(4, 2, 128, -1).transpose(2, 0, 1, 3)
        )

    # permute the M (output-row) dim of the q/k projection weights so the
    # PSUM rows land in DR-plane order per pair: [e0:32|o0:32|e32:64|o32:64]
    # (e = even head dims, o = odd head dims of the pair)
    qk_perm = np.concatenate(
        [b * 128 + np.r_[0:32, 64:96, 32:64, 96:128] for b in range(2)]
    )

    in_maps = []
    for c in range(N_CORES):
        b, hg = c // 4, c % 4
        sl = slice(256 * hg, 256 * hg + 256)
        xTb = x[b].T  # [C, T]
        xq8 = dr_pack(xTb.astype(f8))  # [128, 4, 2, T]
        in_maps.append(
            {
                # per-partition-contiguous packings (cheap DMA descriptors)
                "xc": np.ascontiguousarray(
                    xTb.astype(bf).reshape(8, 128, 4, 512).transpose(1, 2, 0, 3)
                ),
                "xq8": np.ascontiguousarray(
                    xq8.reshape(128, 4, 2, 4, 512).transpose(0, 3, 1, 2, 4)
                ),
                "wqk8": np.ascontiguousarray(np.stack([
                    dr_pack((Wq[sl, :].T * W8SCALE).astype(f8)[:, qk_perm]),
                    dr_pack((Wk[sl, :].T * W8SCALE).astype(f8)[:, qk_perm]),
                ], axis=1)),
                "wv": np.ascontiguousarray(
                    Wv[sl, :].T.astype(bf).reshape(8, 128, 256).transpose(1, 0, 2)
                ),
                "wo": np.ascontiguousarray(
                    Wo[:, sl].T.astype(bf).reshape(2, 128, 1024).transpose(1, 0, 2)
                ),
            }
        )
    return in_maps


def run_spmd(in_maps, trace=False, repeats=1, **kw):
    nc = build_kernel(repeats)
    return run_bass_kernel_spmd(nc, in_maps, list(range(N_CORES)), trace=trace, **kw)


def gather(results, bo):
    bo = np.asarray(bo, dtype=np.float32)
    y = np.empty((B, T, C), dtype=np.float32)
    for b in range(B):
        acc = results[4 * b]["y"].astype(np.float32).copy()
        for g in range(1, 4):
            acc += results[4 * b + g]["y"].astype(np.float32)
        y[b] = acc + bo[None, :]
    return y


def kernel(x, Wq, Wk, Wv, Wo, bo):
    res = run_spmd(make_in_maps(x, Wq, Wk, Wv, Wo))
    return gather(res.results, bo)



# revision 98
# speedup vs baseline: 1.0023x; 1.0023x over previous
"""Causal self-attention (B=2, T=2048, C=1024, H=16, D=64) on 8 trn2 cores.

Sharding: core c handles batch b = c//4 and head group hg = c%4 (heads
4*hg .. 4*hg+3).  Each core computes q/k/v projections for its 4 heads,
causal-softmax attention, and a partial output projection
y_partial = O_heads @ Wo[:, heads].T.  The host sums the 4 partials per
batch and adds the bias.

Numerics (measured 1.21e-2 max-rel vs the 2e-2 gate):
  - q/k projections run in fp8e4m3 with perf_mode=DoubleRow (K=256 per
    matmul): weights pre-scaled by 16 (folded back out via the exp
    scale), activations straight-cast.
  - S = k^T q ALSO runs fp8 DoubleRow (half PE cost per column): the
    projection PSUM is re-quantized to e4m3 and repacked into the
    DoubleRow operand layout (see below).  Double quantization of q,k
    raises max-rel from ~0.9e-2 to ~1.2e-2.
  - V/output projections, P, O use bf16 operands with fp32 PSUM.

Layout:
  q/k DR operands: the projection weights' M columns are host-permuted
          to [e0:32|o0:32|e32:64|o32:64] per pair, so the projection
          PSUM rows land in DoubleRow *plane* order.  Per (pair, chunk):
          one DVE tensor_scalar copy PSUM -> qkf8 [128, 1024] f8
          (q cols 0:512, k 512:1024), then two partition-block DMAs
          fold it into qkdr [64, 2, 1024]: partition p<32 head-even,
          32:64 head-odd; ko plane 0 = dims 0:32, plane 1 = 32:64.
          S matmuls then use lhsT/rhs [32, 2, cols] with Ki=32, K=64.
  V is projected PRE-TRANSPOSED (lhsT = xc tile, rhs = wv tile) into
          [128 Tk, 256] PSUM, then one strided copy per Tk tile into
          vaug[t] [128, 260] = [Vh0|1|Vh1|1|Vh2|1|Vh3|1]; the ones
          columns (memset once) make PSUM row 64 of the O^T
          accumulation the softmax denominator
  S^T_j = kdr_j^T @ qdr_I  (fp8 DR, k-major, causally trimmed)
  P = exp(S/(sqrt(C)*256)) on ACT, batched [128,1024] per head-group;
          the 2-act diagonal split is merged into ONE strided 3D-AP act
          (over-computing bounded stale cols nothing reads); the
          boundary strip is masked on DVE
  O^T_h = sum_j vaug[j][:, 65h:65h+65].T @ P_j  -> [65, 512] PSUM
  normalize: reciprocal (DVE) -> partition_broadcast (GPSIMD) -> mul;
          head-odd half shifted to partitions 64:128 via SBUF DMA --
          processed h=1 FIRST since its chain is one DMA longer and
          gates the y units (worth ~1.5us over the 16 instances)
  y: per (Tk, 512-col) unit, 2 matmuls (K=256 over the head pairs) ->
          bf16 copy into a persistent per-chunk staging tile -> one
          128-row DMA per Tk tile

Emission / scheduling: the Tile scheduler is a readiness+priority list
scheduler; the per-engine streams it execute in order.  With S in fp8
the kernel is ACT(exp)-bound (~86us ACT vs ~79us PE busy), so the
emission keeps the exp stream fed: next-chunk projection groups and
y-projection units are interleaved into the attention stream as PE
filler, paced by an ACT-vs-PE deficit tracker (pay_debt, pop-capped so
a debt spike cannot push the next S group past one exp).  DMA rules
learned from the timeline model: every DRAM tensor is host-packed
per-partition-contiguous (strided DRAM APs cost 1-4us of descriptor
generation on the issuing sequencer); HWDGE issue is ~626ns serial per
DMA and the DMA device runs one transfer at a time, so the lead-in
keeps the pre-fold DMA count minimal and later chunks' x transfers are
emitted lazily at the previous chunk's attention start; a DMA issue
seq-WAITS on its input dependency while holding its queue, so the fold
DMAs live on the sync queue (never ahead of latency-critical exp
issues on the ACT queue); chunk-0 k-copies ride the then-idle ACT
engine and chunk-0 V t1..t3 are deferred into the attention stream so
the first S/exp is not queued behind them on the in-order PE stream.
"""
import numpy as np
import ml_dtypes

import concourse.tile as tile
import concourse.mybir as mybir
from concourse import bacc
from concourse.bass_utils import run_bass_kernel_spmd

FP = mybir.dt.float32
BF = mybir.dt.bfloat16
F8 = mybir.dt.float8e4
W8SCALE = 16.0  # fp8 weight pre-scale (folded back out via the exp scale)
NDT = 4  # DoubleRow K-tiles over the embedding dim (4 x (128x2))
B, T, C = 2, 2048, 1024
H, D = 16, 64
SCALE = 1.0 / 32.0  # 1/sqrt(C)
S_SCALE = SCALE / (W8SCALE * W8SCALE)  # q,k carry a 16x pre-scale each
N_CORES = 8
NKT = C // 128  # 8 K-tiles over the embedding dim
NTK = T // 128  # 16 Tk tiles
NI = T // 512  # 4 Tq chunks
EXP = mybir.ActivationFunctionType.Exp

_nc_cache = {}


def build_kernel(repeats=1, hmix=False):
    key = (repeats, hmix)
    if key in _nc_cache:
        return _nc_cache[key]

    nc = bacc.Bacc("TRN2", target_bir_lowering=False, debug=False)

    # all DRAM inputs are host-packed so every DMA reads per-partition
    # CONTIGUOUS bytes: strided DRAM APs cost 1-4us of descriptor
    # generation on the issuing sequencer, which stalled the lead-in
    xc_d = nc.dram_tensor("xc", [128, NI, NKT, 512], BF, kind="ExternalInput").ap()
    xq8_d = nc.dram_tensor("xq8", [128, NI, NDT, 2, 512], F8, kind="ExternalInput").ap()
    wqk8_d = nc.dram_tensor("wqk8", [128, 2, NDT, 2, 256], F8, kind="ExternalInput").ap()
    wv_d = nc.dram_tensor("wv", [128, NKT, 256], BF, kind="ExternalInput").ap()
    wo_d = nc.dram_tensor("wo", [128, 2, C], BF, kind="ExternalInput").ap()
    y_d = nc.dram_tensor("y", [T, C], BF, kind="ExternalOutput").ap()

    # mask[p, i] = 1 iff i >= p : triangular strip at the causal boundary
    mask_np = (
        np.arange(128)[None, :] >= np.arange(128)[:, None]
    ).astype(ml_dtypes.bfloat16)
    mask_d = nc.inline_tensor(mask_np, "mask_tri").ap()

    with tile.TileContext(nc) as tc:
        with (
            tc.tile_pool(name="persist", bufs=1) as pp,
            tc.tile_pool(name="ppool", bufs=20) as ppool,
            tc.tile_pool(name="spool", bufs=4) as spool,
            tc.tile_pool(name="qkf", bufs=4) as qkf_pool,
            tc.tile_pool(name="ps_s", bufs=2, space="PSUM") as ps_s,
            tc.tile_pool(name="ps_o", bufs=2, space="PSUM") as ps_o,
            tc.tile_pool(name="ps_mm", bufs=2, space="PSUM") as ps_mm,
        ):
            # ---- critical-path DMAs first.  HWDGE issue is ~626ns serial
            # per DMA, so the pre-fold lead-in keeps the DMA count minimal:
            # scalar queue [wqk8, wv, folds...], sync queue [xq8[0], xc[0],
            # mask, rest] ----
            wqk8 = pp.tile([128, 2, NDT, 2, 256], F8, tag="wqk8")
            nc.scalar.dma_start(wqk8[:, 0, :, :, :], wqk8_d[:, 0, :, :, :])
            xq8 = [
                pp.tile([128, NDT, 2, 512], F8, tag=f"xq8{c}", name=f"xq8_{c}")
                for c in range(NI)
            ]
            nc.sync.dma_start(xq8[0][:, 0:2, :, :], xq8_d[:, 0, 0:2, :, :])
            nc.scalar.dma_start(wqk8[:, 1, :, :, :], wqk8_d[:, 1, :, :, :])
            nc.sync.dma_start(xq8[0][:, 2:4, :, :], xq8_d[:, 0, 2:4, :, :])
            wv_big = pp.tile([128, NKT, 256], BF, tag="wv")
            nc.scalar.dma_start(wv_big[:, :, :], wv_d[:, :, :])
            # x for the V projection; one contiguous DMA per chunk
            xc = [
                pp.tile([128, NKT, 512], BF, tag=f"xc{c}", name=f"xc{c}")
                for c in range(NI)
            ]
            nc.sync.dma_start(xc[0][:, 0:4, :], xc_d[:, 0, 0:4, :])
            nc.sync.dma_start(xc[0][:, 4:8, :], xc_d[:, 0, 4:8, :])
            mask = pp.tile([128, 128], BF, tag="mask")
            nc.sync.dma_start(mask[:], mask_d[:])
            # x for chunks >= 1 and wo are DMA'd lazily at the start of the
            # PREVIOUS chunk's attention: the DMA device serializes
            # transfers in issue order, and the chunk-c fold DMAs must not
            # queue behind 3 chunks of x traffic (2.9us each)
            wo_big = pp.tile([128, 2, C], BF, tag="wo")

            def emit_late_dmas(c):  # called at start of attention chunk c
                if R[0] > 0:
                    return  # inputs already resident on later repeats
                if c + 1 < NI:
                    nc.sync.dma_start(
                        xq8[c + 1][:, :, :, :], xq8_d[:, c + 1, :, :, :]
                    )
                    nc.sync.dma_start(xc[c + 1][:, :, :], xc_d[:, c + 1, :, :])
                if c == 1:
                    nc.sync.dma_start(wo_big[:, :, :], wo_d[:, :, :])

            # preload the exp table while DMAs stream (saves ~1.3us later)
            warm_in = pp.tile([1, 2], FP, tag="warm_in")
            warm_out = pp.tile([1, 2], FP, tag="warm_out")
            nc.vector.memset(warm_in[:], 0.0)
            nc.scalar.activation(warm_out[:], warm_in[:], EXP, scale=SCALE)

            # ---- persistent activations ----
            # qkf8[pair][c]: [128, 1024] f8 staging (q cols 0:512, k cols
            # 512:1024), rows in DR-plane order [e0:32|o0:32|e32:64|o32:64]
            # (host-permuted W cols).  Folded by 2 DMAs into qkdr[pair][c]
            # [64, 2, 1024]: partition p<32 head-even, 32:64 head-odd; ko
            # plane 0 = dims 0:32, plane 1 = dims 32:64.  S matmuls then run
            # fp8 DoubleRow (Ki=32, K=64) at half PE cost.
            qkf8 = [[None] * NI for _ in range(2)]
            qkdr = [
                [pp.tile([64, 2, 1024], F8, tag=f"qkd{p}_{i}", name=f"qkd{p}_{i}") for i in range(NI)]
                for p in range(2)
            ]
            otstc = [
                [pp.tile([128, 512], BF, tag=f"ot{p}_{i}", name=f"ot{p}_{i}") for i in range(NI)]
                for p in range(2)
            ]
            # vaug[t] = [Vh0|1|Vh1|1|Vh2|1|Vh3|1]; ones set once, V cols
            # rewritten per repeat by the strided copy from the V psum
            vaug = [pp.tile([128, 260], BF, tag=f"va{t}", name=f"va{t}") for t in range(NTK)]
            for t in range(NTK):
                nc.vector.memset(vaug[t][:, 64:260:65], 1.0)

            R = [0]

            # ---- PE filler machinery: queues of (cost_ns, emit_fn).
            # proj fillers must all land before the next attention chunk;
            # yproj fillers may linger until the final drain ----
            fillers_proj = []
            fillers_y = []
            debt = [0.0]
            y_reserve = [0]

            def pay_debt(max_pops=3, allow_proj=True):
                # cap pops per call so a debt spike cannot push the next
                # attention group's S matmuls out by more than ~one exp.
                # allow_proj=False while the next chunk's x transfers are
                # still in flight: a popped projection matmul would wait on
                # them IN the in-order PE stream, stalling attention behind
                while max_pops > 0 and debt[0] > 0.0 and (
                    (fillers_proj and allow_proj)
                    or len(fillers_y) > y_reserve[0]
                ):
                    q = (
                        fillers_proj
                        if (fillers_proj and allow_proj)
                        else fillers_y
                    )
                    cost, fn = q.pop(0)
                    fn()
                    debt[0] -= cost
                    max_pops -= 1

            def drain_proj_fillers():
                while fillers_proj:
                    _, fn = fillers_proj.pop(0)
                    fn()
                debt[0] = 0.0

            def drain_y_fillers(on_act=False):
                while fillers_y:
                    _, fn = fillers_y.pop(0)
                    fn(on_act=on_act)

            def drain_all_fillers():
                drain_proj_fillers()
                drain_y_fillers(on_act="tail")

            # ---- emission units ----
            def emit_qk_group(c, nm, pair, copy_on_act=False):
                qk = 0 if nm == "q" else 1
                ps = ps_mm.tile([128, 512], FP, tag="mm",
                                name=f"ps{nm}{c}_{pair}_r{R[0]}")
                for dt in range(NDT):
                    nc.tensor.matmul(
                        ps[:],
                        lhsT=wqk8[:, qk, dt, :, pair * 128 : pair * 128 + 128],
                        rhs=xq8[c][:, dt, :, :],
                        start=(dt == 0),
                        stop=(dt == NDT - 1),
                        perf_mode=mybir.MatmulPerfMode.DoubleRow,
                    )
                # this copy gates the next chunk's whole attention stream;
                # schedule it ahead of other queued DVE work
                if nm == "q":
                    qkf8[pair][c] = qkf_pool.tile(
                        [128, 1024], F8, tag="qkf", name=f"qkf{pair}_{c}_r{R[0]}"
                    )
                half = slice(0, 512) if nm == "q" else slice(512, 1024)
                with tc.high_priority(60):
                    if copy_on_act:
                        nc.scalar.copy(qkf8[pair][c][:, half], ps[:])
                    else:
                        nc.vector.tensor_scalar_mul(
                            qkf8[pair][c][:, half], ps[:], 1.0
                        )
                if nm == "k":  # both halves staged -> fold into DR layout.
                    # sync queue: a DMA issue seq-WAITS on its input dep
                    # while holding its queue, so folds must not share a
                    # queue with latency-critical issues (exp is on ACT)
                    with tc.high_priority(60):
                        nc.sync.dma_start(
                            qkdr[pair][c][:, 0, :], qkf8[pair][c][0:64, :]
                        )
                        nc.sync.dma_start(
                            qkdr[pair][c][:, 1, :], qkf8[pair][c][64:128, :]
                        )

            def emit_v_group(t):
                c, ts = t // 4, t % 4
                ps = ps_mm.tile([128, 512], FP, tag="mm",
                                name=f"psv{t}_r{R[0]}")
                for kk in range(NKT):
                    nc.tensor.matmul(
                        ps[:, 0:256],
                        lhsT=xc[c][:, kk, ts * 128 : ts * 128 + 128],
                        rhs=wv_big[:, kk, :],
                        start=(kk == 0),
                        stop=(kk == NKT - 1),
                    )
                with tc.high_priority(60):
                    nc.vector.tensor_copy(
                        vaug[t][:].rearrange("p (n d) -> p n d", n=4)[:, :, 0:64],
                        ps[:, 0:256].rearrange("p (n d) -> p n d", n=4),
                    )

            def emit_proj_chunk(c):
                # k-copies ride the (idle) ACT engine at the lead-in so the
                # fold does not wait behind the q-copy on DVE; V t1..t3 are
                # deferred into the attention stream so the first S/exp is
                # not queued behind 24 V matmuls on the in-order PE stream
                on_act = R[0] == 0
                for pair in range(2):
                    emit_qk_group(c, "q", pair)
                    emit_qk_group(c, "k", pair, copy_on_act=on_act)
                emit_v_group(4 * c)
                for t in range(4 * c + 1, 4 * c + 4):
                    fillers_proj.insert(
                        t - 4 * c - 1, (870, lambda t=t: emit_v_group(t))
                    )

            def proj_chunk_fillers(c):
                for pair in range(2):
                    fillers_proj.append((430, lambda pair=pair: emit_qk_group(
                        c, "q", pair)))
                    fillers_proj.append((430, lambda pair=pair: emit_qk_group(
                        c, "k", pair)))
                for t in range(4 * c, 4 * c + 4):
                    fillers_proj.append((870, lambda t=t: emit_v_group(t)))

            yt_chunk = [
                pp.tile([128, 4, 1024], BF, tag=f"yc{c}", name=f"yc{c}")
                for c in range(NI)
            ]
            y_done = {}

            def emit_yproj_unit(t, nch, on_act=False):
                ps = ps_mm.tile([128, 512], FP, tag="mm",
                                name=f"psy{t}_{nch}_r{R[0]}")
                for pair in range(2):
                    nc.tensor.matmul(
                        ps[:],
                        lhsT=otstc[pair][t // 4][
                            :, (t % 4) * 128 : (t % 4) * 128 + 128
                        ],
                        rhs=wo_big[:, pair, nch * 512 : nch * 512 + 512],
                        start=(pair == 0),
                        stop=(pair == 1),
                    )
                c = t // 4
                dst = yt_chunk[c][:, t % 4, nch * 512 : nch * 512 + 512]
                # mid-run copies go to DVE (ACT is exp-saturated); in the
                # reserve drain the DVE queue is full of normalize work so
                # use ACT; at the tail both are idle, so alternate
                if on_act == "act" or (
                    on_act == "tail" and (2 * t + nch) % 2 == 0
                ):
                    nc.scalar.copy(dst, ps[:])
                else:
                    nc.vector.tensor_copy(dst, ps[:])
                y_done[c] = y_done.get(c, 0) + 1
                if y_done[c] in (2, 4, 6, 8):  # quarter-chunk -> DMA it
                    lo = y_done[c] // 2 - 1
                    nc.sync.dma_start(
                        y_d[c * 512 + lo * 128 : c * 512 + lo * 128 + 128, :]
                        .rearrange("(n p) d -> p n d", p=128),
                        yt_chunk[c][:, lo : lo + 1, :],
                    )
                    if y_done[c] == 8:
                        y_done[c] = 0

            def emit_attention(I, last=False, pairs=(0, 1)):
                emit_attention_body(I, last, pairs)
                if 1 in pairs:
                    for t in range(4 * I, 4 * I + 4):
                        for nch in range(2):
                            fillers_y.append(
                                (430,
                                 lambda t=t, nch=nch, **kw: emit_yproj_unit(
                                     t, nch, **kw))
                            )

            def emit_attention_body(I, last, pairs):
                jmax = 4 * I + 4
                for pair in pairs:
                    oT = [
                        ps_o.tile([65, 512], FP, tag="oT",
                                  name=f"o{I}_{pair}_{h}_r{R[0]}")
                        for h in range(2)
                    ]

                    def emit_o(g, tiles):
                        j0 = 2 * g
                        for h in range(2):
                            p_sb = tiles[h]
                            head = 2 * pair + h
                            for dj in range(2):
                                j = j0 + dj
                                z = max(0, j * 128 - I * 512)
                                nc.tensor.matmul(
                                    oT[h][:, z:512],
                                    lhsT=vaug[j][:, 65 * head : 65 * head + 65],
                                    rhs=p_sb[:, dj * 512 + z : dj * 512 + 512],
                                    start=(j == 0),
                                    stop=(j == jmax - 1),
                                )

                    prev = None
                    for g in range(jmax // 2):
                        j0 = 2 * g
                        diag = j0 >= 4 * I
                        zs = [max(0, (j0 + dj) * 128 - I * 512) for dj in range(2)]
                        cur = []
                        act_ns = 0.0
                        cols = 0
                        for h in range(2):
                            hsl = slice(32 * h, 32 * h + 32)
                            s_ps = ps_s.tile([128, 1024], FP, tag="s",
                                             name=f"s{I}_{pair}_{h}_{g}_r{R[0]}")
                            for dj in range(2):
                                j = j0 + dj
                                z = zs[dj]
                                kof = 512 + (j % 4) * 128
                                nc.tensor.matmul(
                                    s_ps[:, dj * 512 + z : dj * 512 + 512],
                                    lhsT=qkdr[pair][j // 4][
                                        hsl, :, kof : kof + 128
                                    ],
                                    rhs=qkdr[pair][I][hsl, :, z:512],
                                    start=True,
                                    stop=True,
                                    perf_mode=mybir.MatmulPerfMode.DoubleRow,
                                )
                                cols += 512 - z
                            p_sb = ppool.tile([128, 1024], BF, tag="p",
                                              name=f"p{I}_{pair}_{h}_{g}_r{R[0]}")
                            if not diag or zs[0] == 0:
                                # diag group with z0=0: one act over the whole
                                # tile; the uncomputed gap [512:512+z1] holds
                                # stale S values, bounded so exp stays finite,
                                # and nothing downstream reads it
                                nc.scalar.activation(p_sb[:], s_ps[:], EXP,
                                                     scale=S_SCALE)
                                act_ns += 1024 * 0.833 + 185
                            else:
                                # one strided act at the smaller z: the
                                # over-covered [512+z0:512+z1] region holds
                                # bounded stale S (exp stays finite) and is
                                # never read downstream
                                nc.scalar.activation(
                                    p_sb[:].rearrange(
                                        "p (d t) -> p d t", d=2
                                    )[:, :, zs[0] : 512],
                                    s_ps[:].rearrange(
                                        "p (d t) -> p d t", d=2
                                    )[:, :, zs[0] : 512],
                                    EXP,
                                    scale=S_SCALE,
                                )
                                act_ns += 2 * (512 - zs[0]) * 0.833 + 185
                            for dj in range(2):
                                j = j0 + dj
                                if j >= 4 * I:
                                    z = zs[dj]
                                    ssl = slice(dj * 512 + z, dj * 512 + z + 128)
                                    nc.vector.tensor_mul(
                                        p_sb[:, ssl], p_sb[:, ssl], mask[:]
                                    )
                            cur.append(p_sb)
                        # ACT-vs-PE deficit for this group: exp time vs the
                        # S (fp8 DR, 0.2083/col) + O (bf16, 0.4167/col) time
                        debt[0] += act_ns - cols * 0.625
                        if prev is not None:
                            emit_o(g - 1, prev)
                        pay_debt()
                        prev = cur
                    emit_o(jmax // 2 - 1, prev)
                    if last and pair == 1:
                        # reserved y units keep PE warm through the final
                        # normalize chain
                        y_reserve[0] = 0
                        drain_y_fillers(on_act="act")
                    # normalize O^T by the PSUM row-64 denominator;
                    # h=1 first: its chain is longer (partition-shift DMA)
                    for h in (1, 0):
                        recip = spool.tile([1, 512], FP, tag="recip",
                                           name=f"rc{I}_{pair}_{h}_r{R[0]}")
                        nc.vector.reciprocal(recip[:], oT[h][64:65, :])
                        bcast = spool.tile([64, 512], FP, tag="bcast",
                                           name=f"bc{I}_{pair}_{h}_r{R[0]}")
                        nc.gpsimd.partition_broadcast(bcast[:], recip[:])
                        if h == 0:
                            nc.vector.tensor_mul(
                                otstc[pair][I][0:64, :], oT[h][0:64, :], bcast[:]
                            )
                        else:
                            onrm = spool.tile([64, 512], BF, tag="onrm",
                                              name=f"on{I}_{pair}_r{R[0]}")
                            nc.vector.tensor_mul(onrm[:], oT[h][0:64, :], bcast[:])
                            # partition shift 0->64 needs a DMA
                            nc.sync.dma_start(otstc[pair][I][64:128, :], onrm[:])


            # ---- main emission.  Attention instructions carry high
            # scheduler priority (they form the serial latency chain:
            # S -> exp -> mask -> O -> normalize); projections and
            # y-projection units are normal priority, so the greedy
            # scheduler slots them into PE whenever attention work is
            # not ready ----
            for rep in range(repeats):
                R[0] = rep
                emit_proj_chunk(0)
                for c in range(NI):
                    emit_late_dmas(c)
                    if c + 1 < NI:
                        proj_chunk_fillers(c + 1)
                    else:
                        y_reserve[0] = 8
                    if c < 2:
                        # early chunks have little attention work: drain
                        # next-chunk projections eagerly so their folds land
                        # before this chunk's exp stream runs dry
                        debt[0] += 3500.0 if c == 0 else 2000.0
                    emit_attention(c, last=(c == NI - 1))
                    drain_proj_fillers()
                y_reserve[0] = 0
                drain_all_fillers()

    nc.compile()
    _nc_cache[key] = nc
    return nc


def make_in_maps(x, Wq, Wk, Wv, Wo):
    x = np.asarray(x, dtype=np.float32)
    Wq = np.asarray(Wq, dtype=np.float32)
    Wk = np.asarray(Wk, dtype=np.float32)
    Wv = np.asarray(Wv, dtype=np.float32)
    Wo = np.asarray(Wo, dtype=np.float32)
    bf = ml_dtypes.bfloat16
    f8 = ml_dtypes.float8_e4m3fn

    def dr_pack(a):  # [C, m] -> [128, NDT, 2, m] with k = 256*dt + ki + 128*ko
        return np.ascontiguousarray(
            a.reshape(4, 2, 128, -1).transpose(2, 0, 1, 3)
        )

    # permute the M (output-row) dim of the q/k projection weights so the
    # PSUM rows land in DR-plane order per pair: [e0:32|o0:32|e32:64|o32:64]
    # (e = even head dims, o = odd head dims of the pair)
    qk_perm = np.concatenate(
        [b * 128 + np.r_[0:32, 64:96, 32:64, 96:128] for b in range(2)]
    )

    in_maps = []
    for c in range(N_CORES):
        b, hg = c // 4, c % 4
        sl = slice(256 * hg, 256 * hg + 256)
        xTb = x[b].T  # [C, T]
        xq8 = dr_pack(xTb.astype(f8))  # [128, 4, 2, T]
        in_maps.append(
            {
                # per-partition-contiguous packings (cheap DMA descriptors)
                "xc": np.ascontiguousarray(
                    xTb.astype(bf).reshape(8, 128, 4, 512).transpose(1, 2, 0, 3)
                ),
                "xq8": np.ascontiguousarray(
                    xq8.reshape(128, 4, 2, 4, 512).transpose(0, 3, 1, 2, 4)
                ),
                "wqk8": np.ascontiguousarray(np.stack([
                    dr_pack((Wq[sl, :].T * W8SCALE).astype(f8)[:, qk_perm]),
                    dr_pack((Wk[sl, :].T * W8SCALE).astype(f8)[:, qk_perm]),
                ], axis=1)),
                "wv": np.ascontiguousarray(
                    Wv[sl, :].T.astype(bf).reshape(8, 128, 256).transpose(1, 0, 2)
                ),
                "wo": np.ascontiguousarray(
                    Wo[:, sl].T.astype(bf).reshape(2, 128, 1024).transpose(1, 0, 2)
                ),
            }
        )
    return in_maps


def run_spmd(in_maps, trace=False, repeats=1, **kw):
    nc = build_kernel(repeats)
    return run_bass_kernel_spmd(nc, in_maps, list(range(N_CORES)), trace=trace, **kw)


def gather(results, bo):
    bo = np.asarray(bo, dtype=np.float32)
    y = np.empty((B, T, C), dtype=np.float32)
    for b in range(B):
        acc = results[4 * b]["y"].astype(np.float32).copy()
        for g in range(1, 4):
            acc += results[4 * b + g]["y"].astype(np.float32)
        y[b] = acc + bo[None, :]
    return y


def kernel(x, Wq, Wk, Wv, Wo, bo):
    res = run_spmd(make_in_maps(x, Wq, Wk, Wv, Wo))
    return gather(res.results, bo)



# revision 99
# speedup vs baseline: 1.0037x; 1.0014x over previous
"""Causal self-attention (B=2, T=2048, C=1024, H=16, D=64) on 8 trn2 cores.

Sharding: core c handles batch b = c//4 and head group hg = c%4 (heads
4*hg .. 4*hg+3).  Each core computes q/k/v projections for its 4 heads,
causal-softmax attention, and a partial output projection
y_partial = O_heads @ Wo[:, heads].T.  The host sums the 4 partials per
batch and adds the bias.

Numerics (measured 1.21e-2 max-rel vs the 2e-2 gate):
  - q/k projections run in fp8e4m3 with perf_mode=DoubleRow (K=256 per
    matmul): weights pre-scaled by 16 (folded back out via the exp
    scale), activations straight-cast.
  - S = k^T q ALSO runs fp8 DoubleRow (half PE cost per column): the
    projection PSUM is re-quantized to e4m3 and repacked into the
    DoubleRow operand layout (see below).  Double quantization of q,k
    raises max-rel from ~0.9e-2 to ~1.2e-2.
  - V/output projections, P, O use bf16 operands with fp32 PSUM.

Layout:
  q/k DR operands: the projection weights' M columns are host-permuted
          to [e0:32|o0:32|e32:64|o32:64] per pair, so the projection
          PSUM rows land in DoubleRow *plane* order.  Per (pair, chunk):
          one DVE tensor_scalar copy PSUM -> qkf8 [128, 1024] f8
          (q cols 0:512, k 512:1024), then two partition-block DMAs
          fold it into qkdr [64, 2, 1024]: partition p<32 head-even,
          32:64 head-odd; ko plane 0 = dims 0:32, plane 1 = 32:64.
          S matmuls then use lhsT/rhs [32, 2, cols] with Ki=32, K=64.
  V is projected PRE-TRANSPOSED (lhsT = xc tile, rhs = wv tile) into
          [128 Tk, 256] PSUM, then one strided copy per Tk tile into
          vaug[t] [128, 260] = [Vh0|1|Vh1|1|Vh2|1|Vh3|1]; the ones
          columns (memset once) make PSUM row 64 of the O^T
          accumulation the softmax denominator
  S^T_j = kdr_j^T @ qdr_I  (fp8 DR, k-major, causally trimmed)
  P = exp(S/(sqrt(C)*256)) on ACT, batched [128,1024] per head-group;
          the 2-act diagonal split is merged into ONE strided 3D-AP act
          (over-computing bounded stale cols nothing reads); the
          boundary strip is masked on DVE
  O^T_h = sum_j vaug[j][:, 65h:65h+65].T @ P_j  -> [65, 512] PSUM
  normalize: reciprocal (DVE) -> partition_broadcast (GPSIMD) -> mul;
          head-odd half shifted to partitions 64:128 via SBUF DMA --
          processed h=1 FIRST since its chain is one DMA longer and
          gates the y units (worth ~1.5us over the 16 instances)
  y: per (Tk, 512-col) unit, 2 matmuls (K=256 over the head pairs) ->
          bf16 copy into a persistent per-chunk staging tile -> one
          128-row DMA per Tk tile

Emission / scheduling: the Tile scheduler is a readiness+priority list
scheduler; the per-engine streams it execute in order.  With S in fp8
the kernel is ACT(exp)-bound (~86us ACT vs ~79us PE busy), so the
emission keeps the exp stream fed: next-chunk projection groups and
y-projection units are interleaved into the attention stream as PE
filler, paced by an ACT-vs-PE deficit tracker (pay_debt, pop-capped so
a debt spike cannot push the next S group past one exp).  DMA rules
learned from the timeline model: every DRAM tensor is host-packed
per-partition-contiguous (strided DRAM APs cost 1-4us of descriptor
generation on the issuing sequencer); HWDGE issue is ~626ns serial per
DMA and the DMA device runs one transfer at a time, so the lead-in
keeps the pre-fold DMA count minimal and later chunks' x transfers are
emitted lazily at the previous chunk's attention start; a DMA issue
seq-WAITS on its input dependency while holding its queue, so the fold
DMAs live on the sync queue (never ahead of latency-critical exp
issues on the ACT queue); chunk-0 k-copies ride the then-idle ACT
engine and chunk-0 V t1..t3 are deferred into the attention stream so
the first S/exp is not queued behind them on the in-order PE stream.
"""
import numpy as np
import ml_dtypes

import concourse.tile as tile
import concourse.mybir as mybir
from concourse import bacc
from concourse.bass_utils import run_bass_kernel_spmd

FP = mybir.dt.float32
BF = mybir.dt.bfloat16
F8 = mybir.dt.float8e4
W8SCALE = 16.0  # fp8 weight pre-scale (folded back out via the exp scale)
NDT = 4  # DoubleRow K-tiles over the embedding dim (4 x (128x2))
B, T, C = 2, 2048, 1024
H, D = 16, 64
SCALE = 1.0 / 32.0  # 1/sqrt(C)
S_SCALE = SCALE / (W8SCALE * W8SCALE)  # q,k carry a 16x pre-scale each
N_CORES = 8
NKT = C // 128  # 8 K-tiles over the embedding dim
NTK = T // 128  # 16 Tk tiles
NI = T // 512  # 4 Tq chunks
EXP = mybir.ActivationFunctionType.Exp

_nc_cache = {}


def build_kernel(repeats=1, hmix=False):
    key = (repeats, hmix)
    if key in _nc_cache:
        return _nc_cache[key]

    nc = bacc.Bacc("TRN2", target_bir_lowering=False, debug=False)

    # all DRAM inputs are host-packed so every DMA reads per-partition
    # CONTIGUOUS bytes: strided DRAM APs cost 1-4us of descriptor
    # generation on the issuing sequencer, which stalled the lead-in
    xc_d = nc.dram_tensor("xc", [128, NI, NKT, 512], BF, kind="ExternalInput").ap()
    xq8_d = nc.dram_tensor("xq8", [128, NI, NDT, 2, 512], F8, kind="ExternalInput").ap()
    wqk8_d = nc.dram_tensor("wqk8", [128, 2, NDT, 2, 256], F8, kind="ExternalInput").ap()
    wv_d = nc.dram_tensor("wv", [128, NKT, 256], BF, kind="ExternalInput").ap()
    wo_d = nc.dram_tensor("wo", [128, 2, C], BF, kind="ExternalInput").ap()
    y_d = nc.dram_tensor("y", [T, C], BF, kind="ExternalOutput").ap()

    # mask[p, i] = 1 iff i >= p : triangular strip at the causal boundary
    mask_np = (
        np.arange(128)[None, :] >= np.arange(128)[:, None]
    ).astype(ml_dtypes.bfloat16)
    mask_d = nc.inline_tensor(mask_np, "mask_tri").ap()

    with tile.TileContext(nc) as tc:
        with (
            tc.tile_pool(name="persist", bufs=1) as pp,
            tc.tile_pool(name="ppool", bufs=20) as ppool,
            tc.tile_pool(name="spool", bufs=4) as spool,
            tc.tile_pool(name="qkf", bufs=4) as qkf_pool,
            tc.tile_pool(name="ps_s", bufs=2, space="PSUM") as ps_s,
            tc.tile_pool(name="ps_o", bufs=2, space="PSUM") as ps_o,
            tc.tile_pool(name="ps_mm", bufs=2, space="PSUM") as ps_mm,
        ):
            # ---- critical-path DMAs first.  HWDGE issue is ~626ns serial
            # per DMA, so the pre-fold lead-in keeps the DMA count minimal:
            # scalar queue [wqk8, wv, folds...], sync queue [xq8[0], xc[0],
            # mask, rest] ----
            wqk8 = pp.tile([128, 2, NDT, 2, 256], F8, tag="wqk8")
            nc.scalar.dma_start(wqk8[:, 0, :, :, :], wqk8_d[:, 0, :, :, :])
            xq8 = [
                pp.tile([128, NDT, 2, 512], F8, tag=f"xq8{c}", name=f"xq8_{c}")
                for c in range(NI)
            ]
            nc.sync.dma_start(xq8[0][:, 0:2, :, :], xq8_d[:, 0, 0:2, :, :])
            nc.scalar.dma_start(wqk8[:, 1, :, :, :], wqk8_d[:, 1, :, :, :])
            nc.sync.dma_start(xq8[0][:, 2:4, :, :], xq8_d[:, 0, 2:4, :, :])
            wv_big = pp.tile([128, NKT, 256], BF, tag="wv")
            nc.scalar.dma_start(wv_big[:, :, :], wv_d[:, :, :])
            # x for the V projection; one contiguous DMA per chunk
            xc = [
                pp.tile([128, NKT, 512], BF, tag=f"xc{c}", name=f"xc{c}")
                for c in range(NI)
            ]
            nc.sync.dma_start(xc[0][:, 0:4, :], xc_d[:, 0, 0:4, :])
            nc.sync.dma_start(xc[0][:, 4:8, :], xc_d[:, 0, 4:8, :])
            mask = pp.tile([128, 128], BF, tag="mask")
            nc.sync.dma_start(mask[:], mask_d[:])
            # x for chunks >= 1 and wo are DMA'd lazily at the start of the
            # PREVIOUS chunk's attention: the DMA device serializes
            # transfers in issue order, and the chunk-c fold DMAs must not
            # queue behind 3 chunks of x traffic (2.9us each)
            wo_big = pp.tile([128, 2, C], BF, tag="wo")

            def emit_late_dmas(c):  # called at start of attention chunk c
                if R[0] > 0:
                    return  # inputs already resident on later repeats
                if c + 1 < NI:
                    nc.sync.dma_start(
                        xq8[c + 1][:, :, :, :], xq8_d[:, c + 1, :, :, :]
                    )
                    nc.sync.dma_start(xc[c + 1][:, :, :], xc_d[:, c + 1, :, :])
                if c == 1:
                    nc.sync.dma_start(wo_big[:, :, :], wo_d[:, :, :])

            # preload the exp table while DMAs stream (saves ~1.3us later)
            warm_in = pp.tile([1, 2], FP, tag="warm_in")
            warm_out = pp.tile([1, 2], FP, tag="warm_out")
            nc.vector.memset(warm_in[:], 0.0)
            nc.scalar.activation(warm_out[:], warm_in[:], EXP, scale=SCALE)

            # ---- persistent activations ----
            # qkf8[pair][c]: [128, 1024] f8 staging (q cols 0:512, k cols
            # 512:1024), rows in DR-plane order [e0:32|o0:32|e32:64|o32:64]
            # (host-permuted W cols).  Folded by 2 DMAs into qkdr[pair][c]
            # [64, 2, 1024]: partition p<32 head-even, 32:64 head-odd; ko
            # plane 0 = dims 0:32, plane 1 = dims 32:64.  S matmuls then run
            # fp8 DoubleRow (Ki=32, K=64) at half PE cost.
            qkf8 = [[None] * NI for _ in range(2)]
            qkdr = [
                [pp.tile([64, 2, 1024], F8, tag=f"qkd{p}_{i}", name=f"qkd{p}_{i}") for i in range(NI)]
                for p in range(2)
            ]
            otstc = [
                [pp.tile([128, 512], BF, tag=f"ot{p}_{i}", name=f"ot{p}_{i}") for i in range(NI)]
                for p in range(2)
            ]
            # vaug[t] = [Vh0|1|Vh1|1|Vh2|1|Vh3|1]; ones set once, V cols
            # rewritten per repeat by the strided copy from the V psum
            vaug = [pp.tile([128, 260], BF, tag=f"va{t}", name=f"va{t}") for t in range(NTK)]
            for t in range(NTK):
                nc.vector.memset(vaug[t][:, 64:260:65], 1.0)

            R = [0]

            # ---- PE filler machinery: queues of (cost_ns, emit_fn).
            # proj fillers must all land before the next attention chunk;
            # yproj fillers may linger until the final drain ----
            fillers_proj = []
            fillers_y = []
            debt = [0.0]
            y_reserve = [0]

            def pay_debt(max_pops=3, allow_proj=True):
                # cap pops per call so a debt spike cannot push the next
                # attention group's S matmuls out by more than ~one exp.
                # allow_proj=False while the next chunk's x transfers are
                # still in flight: a popped projection matmul would wait on
                # them IN the in-order PE stream, stalling attention behind
                while max_pops > 0 and debt[0] > 0.0 and (
                    (fillers_proj and allow_proj)
                    or len(fillers_y) > y_reserve[0]
                ):
                    q = (
                        fillers_proj
                        if (fillers_proj and allow_proj)
                        else fillers_y
                    )
                    cost, fn = q.pop(0)
                    fn()
                    debt[0] -= cost
                    max_pops -= 1

            def drain_proj_fillers():
                while fillers_proj:
                    _, fn = fillers_proj.pop(0)
                    fn()
                debt[0] = 0.0

            def drain_y_fillers(on_act=False):
                while fillers_y:
                    _, fn = fillers_y.pop(0)
                    fn(on_act=on_act)

            def drain_all_fillers():
                drain_proj_fillers()
                drain_y_fillers(on_act="tail")

            # ---- emission units ----
            def emit_qk_group(c, nm, pair, copy_on_act=False):
                qk = 0 if nm == "q" else 1
                ps = ps_mm.tile([128, 512], FP, tag="mm",
                                name=f"ps{nm}{c}_{pair}_r{R[0]}")
                for dt in range(NDT):
                    nc.tensor.matmul(
                        ps[:],
                        lhsT=wqk8[:, qk, dt, :, pair * 128 : pair * 128 + 128],
                        rhs=xq8[c][:, dt, :, :],
                        start=(dt == 0),
                        stop=(dt == NDT - 1),
                        perf_mode=mybir.MatmulPerfMode.DoubleRow,
                    )
                # this copy gates the next chunk's whole attention stream;
                # schedule it ahead of other queued DVE work
                if nm == "q":
                    qkf8[pair][c] = qkf_pool.tile(
                        [128, 1024], F8, tag="qkf", name=f"qkf{pair}_{c}_r{R[0]}"
                    )
                half = slice(0, 512) if nm == "q" else slice(512, 1024)
                with tc.high_priority(60):
                    if copy_on_act:
                        nc.scalar.copy(qkf8[pair][c][:, half], ps[:])
                    else:
                        nc.vector.tensor_scalar_mul(
                            qkf8[pair][c][:, half], ps[:], 1.0
                        )
                if nm == "k":  # both halves staged -> fold into DR layout.
                    # sync queue: a DMA issue seq-WAITS on its input dep
                    # while holding its queue, so folds must not share a
                    # queue with latency-critical issues (exp is on ACT)
                    with tc.high_priority(60):
                        nc.sync.dma_start(
                            qkdr[pair][c][:, 0, :], qkf8[pair][c][0:64, :]
                        )
                        nc.sync.dma_start(
                            qkdr[pair][c][:, 1, :], qkf8[pair][c][64:128, :]
                        )

            def emit_v_group(t):
                c, ts = t // 4, t % 4
                ps = ps_mm.tile([128, 512], FP, tag="mm",
                                name=f"psv{t}_r{R[0]}")
                for kk in range(NKT):
                    nc.tensor.matmul(
                        ps[:, 0:256],
                        lhsT=xc[c][:, kk, ts * 128 : ts * 128 + 128],
                        rhs=wv_big[:, kk, :],
                        start=(kk == 0),
                        stop=(kk == NKT - 1),
                    )
                with tc.high_priority(60):
                    nc.vector.tensor_copy(
                        vaug[t][:].rearrange("p (n d) -> p n d", n=4)[:, :, 0:64],
                        ps[:, 0:256].rearrange("p (n d) -> p n d", n=4),
                    )

            def emit_proj_chunk(c):
                # k-copies ride the (idle) ACT engine at the lead-in so the
                # fold does not wait behind the q-copy on DVE; V t1..t3 are
                # deferred into the attention stream so the first S/exp is
                # not queued behind 24 V matmuls on the in-order PE stream
                on_act = R[0] == 0
                for pair in range(2):
                    emit_qk_group(c, "q", pair)
                    emit_qk_group(c, "k", pair, copy_on_act=on_act)
                emit_v_group(4 * c)
                for t in range(4 * c + 1, 4 * c + 4):
                    fillers_proj.insert(
                        t - 4 * c - 1, (870, lambda t=t: emit_v_group(t))
                    )

            def proj_chunk_fillers(c):
                for pair in range(2):
                    fillers_proj.append((430, lambda pair=pair: emit_qk_group(
                        c, "q", pair)))
                    fillers_proj.append((430, lambda pair=pair: emit_qk_group(
                        c, "k", pair)))
                for t in range(4 * c, 4 * c + 4):
                    fillers_proj.append((870, lambda t=t: emit_v_group(t)))

            yt_chunk = [
                pp.tile([128, 4, 1024], BF, tag=f"yc{c}", name=f"yc{c}")
                for c in range(NI)
            ]
            y_done = {}

            def emit_yproj_unit(t, nch, on_act=False):
                ps = ps_mm.tile([128, 512], FP, tag="mm",
                                name=f"psy{t}_{nch}_r{R[0]}")
                for pair in range(2):
                    nc.tensor.matmul(
                        ps[:],
                        lhsT=otstc[pair][t // 4][
                            :, (t % 4) * 128 : (t % 4) * 128 + 128
                        ],
                        rhs=wo_big[:, pair, nch * 512 : nch * 512 + 512],
                        start=(pair == 0),
                        stop=(pair == 1),
                    )
                c = t // 4
                dst = yt_chunk[c][:, t % 4, nch * 512 : nch * 512 + 512]
                # mid-run copies go to DVE (ACT is exp-saturated); in the
                # reserve drain the DVE queue is full of normalize work so
                # use ACT; at the tail both are idle, so alternate
                if on_act == "act" or (
                    on_act == "tail" and (2 * t + nch) % 2 == 0
                ):
                    nc.scalar.copy(dst, ps[:])
                else:
                    nc.vector.tensor_copy(dst, ps[:])
                y_done[c] = y_done.get(c, 0) + 1
                if y_done[c] in (2, 4, 6, 8):  # quarter-chunk -> DMA it
                    lo = y_done[c] // 2 - 1
                    nc.sync.dma_start(
                        y_d[c * 512 + lo * 128 : c * 512 + lo * 128 + 128, :]
                        .rearrange("(n p) d -> p n d", p=128),
                        yt_chunk[c][:, lo : lo + 1, :],
                    )
                    if y_done[c] == 8:
                        y_done[c] = 0

            def emit_attention(I, last=False, pairs=(0, 1)):
                emit_attention_body(I, last, pairs)
                if 1 in pairs:
                    for t in range(4 * I, 4 * I + 4):
                        for nch in range(2):
                            fillers_y.append(
                                (430,
                                 lambda t=t, nch=nch, **kw: emit_yproj_unit(
                                     t, nch, **kw))
                            )

            def emit_attention_body(I, last, pairs):
                jmax = 4 * I + 4
                for pair in pairs:
                    oT = [
                        ps_o.tile([65, 512], FP, tag="oT",
                                  name=f"o{I}_{pair}_{h}_r{R[0]}")
                        for h in range(2)
                    ]

                    def emit_o(g, tiles):
                        j0 = 2 * g
                        for h in range(2):
                            p_sb = tiles[h]
                            head = 2 * pair + h
                            for dj in range(2):
                                j = j0 + dj
                                z = max(0, j * 128 - I * 512)
                                nc.tensor.matmul(
                                    oT[h][:, z:512],
                                    lhsT=vaug[j][:, 65 * head : 65 * head + 65],
                                    rhs=p_sb[:, dj * 512 + z : dj * 512 + 512],
                                    start=(j == 0),
                                    stop=(j == jmax - 1),
                                )

                    prev = None
                    for g in range(jmax // 2):
                        j0 = 2 * g
                        diag = j0 >= 4 * I
                        zs = [max(0, (j0 + dj) * 128 - I * 512) for dj in range(2)]
                        cur = []
                        act_ns = 0.0
                        cols = 0
                        for h in range(2):
                            hsl = slice(32 * h, 32 * h + 32)
                            s_ps = ps_s.tile([128, 1024], FP, tag="s",
                                             name=f"s{I}_{pair}_{h}_{g}_r{R[0]}")
                            for dj in range(2):
                                j = j0 + dj
                                z = zs[dj]
                                kof = 512 + (j % 4) * 128
                                nc.tensor.matmul(
                                    s_ps[:, dj * 512 + z : dj * 512 + 512],
                                    lhsT=qkdr[pair][j // 4][
                                        hsl, :, kof : kof + 128
                                    ],
                                    rhs=qkdr[pair][I][hsl, :, z:512],
                                    start=True,
                                    stop=True,
                                    perf_mode=mybir.MatmulPerfMode.DoubleRow,
                                )
                                cols += 512 - z
                            p_sb = ppool.tile([128, 1024], BF, tag="p",
                                              name=f"p{I}_{pair}_{h}_{g}_r{R[0]}")
                            if not diag or zs[0] == 0:
                                # diag group with z0=0: one act over the whole
                                # tile; the uncomputed gap [512:512+z1] holds
                                # stale S values, bounded so exp stays finite,
                                # and nothing downstream reads it
                                nc.scalar.activation(p_sb[:], s_ps[:], EXP,
                                                     scale=S_SCALE)
                                act_ns += 1024 * 0.833 + 185
                            else:
                                # one strided act at the smaller z: the
                                # over-covered [512+z0:512+z1] region holds
                                # bounded stale S (exp stays finite) and is
                                # never read downstream
                                nc.scalar.activation(
                                    p_sb[:].rearrange(
                                        "p (d t) -> p d t", d=2
                                    )[:, :, zs[0] : 512],
                                    s_ps[:].rearrange(
                                        "p (d t) -> p d t", d=2
                                    )[:, :, zs[0] : 512],
                                    EXP,
                                    scale=S_SCALE,
                                )
                                act_ns += 2 * (512 - zs[0]) * 0.833 + 185
                            for dj in range(2):
                                j = j0 + dj
                                if j >= 4 * I:
                                    z = zs[dj]
                                    ssl = slice(dj * 512 + z, dj * 512 + z + 128)
                                    nc.vector.tensor_mul(
                                        p_sb[:, ssl], p_sb[:, ssl], mask[:]
                                    )
                            cur.append(p_sb)
                        # ACT-vs-PE deficit for this group: exp time vs the
                        # S (fp8 DR, 0.2083/col) + O (bf16, 0.4167/col) time
                        debt[0] += act_ns - cols * 0.625
                        if prev is not None:
                            emit_o(g - 1, prev)
                        pay_debt()
                        prev = cur
                    emit_o(jmax // 2 - 1, prev)
                    if last and pair == 1:
                        # reserved y units keep PE warm through the final
                        # normalize chain
                        y_reserve[0] = 0
                        drain_y_fillers(on_act="act")
                    # normalize O^T by the PSUM row-64 denominator;
                    # h=1 first: its chain is longer (partition-shift DMA)
                    for h in (1, 0):
                        recip = spool.tile([1, 512], FP, tag="recip",
                                           name=f"rc{I}_{pair}_{h}_r{R[0]}")
                        nc.vector.reciprocal(recip[:], oT[h][64:65, :])
                        bcast = spool.tile([64, 512], FP, tag="bcast",
                                           name=f"bc{I}_{pair}_{h}_r{R[0]}")
                        nc.gpsimd.partition_broadcast(bcast[:], recip[:])
                        if h == 0:
                            nc.vector.tensor_mul(
                                otstc[pair][I][0:64, :], oT[h][0:64, :], bcast[:]
                            )
                        else:
                            onrm = spool.tile([64, 512], BF, tag="onrm",
                                              name=f"on{I}_{pair}_r{R[0]}")
                            nc.vector.tensor_mul(onrm[:], oT[h][0:64, :], bcast[:])
                            # partition shift 0->64 needs a DMA
                            nc.sync.dma_start(otstc[pair][I][64:128, :], onrm[:])


            # ---- main emission.  Attention instructions carry high
            # scheduler priority (they form the serial latency chain:
            # S -> exp -> mask -> O -> normalize); projections and
            # y-projection units are normal priority, so the greedy
            # scheduler slots them into PE whenever attention work is
            # not ready ----
            for rep in range(repeats):
                R[0] = rep
                emit_proj_chunk(0)
                for c in range(NI):
                    emit_late_dmas(c)
                    if c + 1 < NI:
                        proj_chunk_fillers(c + 1)
                    else:
                        y_reserve[0] = 8
                    if c < 2:
                        # early chunks have little attention work: drain
                        # next-chunk projections eagerly so their folds land
                        # before this chunk's exp stream runs dry
                        debt[0] += 3500.0 if c == 0 else 1500.0
                    emit_attention(c, last=(c == NI - 1))
                    drain_proj_fillers()
                y_reserve[0] = 0
                drain_all_fillers()

    nc.compile()
    _nc_cache[key] = nc
    return nc


def make_in_maps(x, Wq, Wk, Wv, Wo):
    x = np.asarray(x, dtype=np.float32)
    Wq = np.asarray(Wq, dtype=np.float32)
    Wk = np.asarray(Wk, dtype=np.float32)
    Wv = np.asarray(Wv, dtype=np.float32)
    Wo = np.asarray(Wo, dtype=np.float32)
    bf = ml_dtypes.bfloat16
    f8 = ml_dtypes.float8_e4m3fn

    def dr_pack(a):  # [C, m] -> [128, NDT, 2, m] with k = 256*dt + ki + 128*ko
        return np.ascontiguousarray(
            a.reshape(4, 2, 128, -1).transpose(2, 0, 1, 3)
        )

    # permute the M (output-row) dim of the q/k projection weights so the
    # PSUM rows land in DR-plane order per pair: [e0:32|o0:32|e32:64|o32:64]
    # (e = even head dims, o = odd head dims of the pair)
    qk_perm = np.concatenate(
        [b * 128 + np.r_[0:32, 64:96, 32:64, 96:128] for b in range(2)]
    )

    in_maps = []
    for c in range(N_CORES):
        b, hg = c // 4, c % 4
        sl = slice(256 * hg, 256 * hg + 256)
        xTb = x[b].T  # [C, T]
        xq8 = dr_pack(xTb.astype(f8))  # [128, 4, 2, T]
        in_maps.append(
            {
                # per-partition-contiguous packings (cheap DMA descriptors)
                "xc": np.ascontiguousarray(
                    xTb.astype(bf).reshape(8, 128, 4, 512).transpose(1, 2, 0, 3)
                ),
                "xq8": np.ascontiguousarray(
                    xq8.reshape(128, 4, 2, 4, 512).transpose(0, 3, 1, 2, 4)
                ),
                "wqk8": np.ascontiguousarray(np.stack([
                    dr_pack((Wq[sl, :].T * W8SCALE).astype(f8)[:, qk_perm]),
                    dr_pack((Wk[sl, :].T * W8SCALE).astype(f8)[:, qk_perm]),
                ], axis=1)),
                "wv": np.ascontiguousarray(
                    Wv[sl, :].T.astype(bf).reshape(8, 128, 256).transpose(1, 0, 2)
                ),
                "wo": np.ascontiguousarray(
                    Wo[:, sl].T.astype(bf).reshape(2, 128, 1024).transpose(1, 0, 2)
                ),
            }
        )
    return in_maps


def run_spmd(in_maps, trace=False, repeats=1, **kw):
    nc = build_kernel(repeats)
    return run_bass_kernel_spmd(nc, in_maps, list(range(N_CORES)), trace=trace, **kw)


def gather(results, bo):
    bo = np.asarray(bo, dtype=np.float32)
    y = np.empty((B, T, C), dtype=np.float32)
    for b in range(B):
        acc = results[4 * b]["y"].astype(np.float32).copy()
        for g in range(1, 4):
            acc += results[4 * b + g]["y"].astype(np.float32)
        y[b] = acc + bo[None, :]
    return y


def kernel(x, Wq, Wk, Wv, Wo, bo):
    res = run_spmd(make_in_maps(x, Wq, Wk, Wv, Wo))
    return gather(res.results, bo)



# revision 100
# speedup vs baseline: 1.0164x; 1.0126x over previous
"""Causal self-attention (B=2, T=2048, C=1024, H=16, D=64) on 8 trn2 cores.

Sharding: core c handles batch b = c//4 and head group hg = c%4 (heads
4*hg .. 4*hg+3).  Each core computes q/k/v projections for its 4 heads,
causal-softmax attention, and a partial output projection
y_partial = O_heads @ Wo[:, heads].T.  The host sums the 4 partials per
batch and adds the bias.

Numerics (measured 1.21e-2 max-rel vs the 2e-2 gate):
  - q/k projections run in fp8e4m3 with perf_mode=DoubleRow (K=256 per
    matmul): weights pre-scaled by 16 (folded back out via the exp
    scale), activations straight-cast.
  - S = k^T q ALSO runs fp8 DoubleRow (half PE cost per column): the
    projection PSUM is re-quantized to e4m3 and repacked into the
    DoubleRow operand layout (see below).  Double quantization of q,k
    raises max-rel from ~0.9e-2 to ~1.2e-2.
  - V/output projections, P, O use bf16 operands with fp32 PSUM.

Layout:
  q/k DR operands: the projection weights' M columns are host-permuted
          to [e0:32|o0:32|e32:64|o32:64] per pair, so the projection
          PSUM rows land in DoubleRow *plane* order.  Per (pair, chunk):
          one DVE tensor_scalar copy PSUM -> qkf8 [128, 1024] f8
          (q cols 0:512, k 512:1024), then two partition-block DMAs
          fold it into qkdr [64, 2, 1024]: partition p<32 head-even,
          32:64 head-odd; ko plane 0 = dims 0:32, plane 1 = 32:64.
          S matmuls then use lhsT/rhs [32, 2, cols] with Ki=32, K=64.
  V is projected PRE-TRANSPOSED (lhsT = xc tile, rhs = wv tile) into
          [128 Tk, 256] PSUM, then one strided copy per Tk tile into
          vaug[t] [128, 260] = [Vh0|1|Vh1|1|Vh2|1|Vh3|1]; the ones
          columns (memset once) make PSUM row 64 of the O^T
          accumulation the softmax denominator
  S^T_j = kdr_j^T @ qdr_I  (fp8 DR, k-major, causally trimmed)
  P = exp(S/(sqrt(C)*256)) on ACT, batched [128,1024] per head-group;
          the 2-act diagonal split is merged into ONE strided 3D-AP act
          (over-computing bounded stale cols nothing reads); the
          boundary strip is masked on DVE
  O^T_h = sum_j vaug[j][:, 65h:65h+65].T @ P_j  -> [65, 512] PSUM
  normalize: reciprocal (DVE) -> partition_broadcast (GPSIMD) -> mul;
          head-odd half shifted to partitions 64:128 via SBUF DMA --
          processed h=1 FIRST since its chain is one DMA longer and
          gates the y units (worth ~1.5us over the 16 instances)
  y: per (Tk, 512-col) unit, 2 matmuls (K=256 over the head pairs) ->
          bf16 copy into a persistent per-chunk staging tile -> one
          128-row DMA per Tk tile

Emission / scheduling: the Tile scheduler is a readiness+priority list
scheduler; the per-engine streams it execute in order.  With S in fp8
the kernel is ACT(exp)-bound (~86us ACT vs ~79us PE busy), so the
emission keeps the exp stream fed: next-chunk projection groups and
y-projection units are interleaved into the attention stream as PE
filler, paced by an ACT-vs-PE deficit tracker (pay_debt, pop-capped so
a debt spike cannot push the next S group past one exp).  DMA rules
learned from the timeline model: every DRAM tensor is host-packed
per-partition-contiguous (strided DRAM APs cost 1-4us of descriptor
generation on the issuing sequencer); HWDGE issue is ~626ns serial per
DMA and the DMA device runs one transfer at a time, so the lead-in
keeps the pre-fold DMA count minimal and later chunks' x transfers are
emitted lazily at the previous chunk's attention start; a DMA issue
seq-WAITS on its input dependency while holding its queue, so the fold
DMAs live on the sync queue (never ahead of latency-critical exp
issues on the ACT queue); chunk-0 k-copies ride the then-idle ACT
engine and chunk-0 V t1..t3 are deferred into the attention stream so
the first S/exp is not queued behind them on the in-order PE stream.
"""
import numpy as np
import ml_dtypes

import concourse.tile as tile
import concourse.mybir as mybir
from concourse import bacc
from concourse.bass_utils import run_bass_kernel_spmd

FP = mybir.dt.float32
BF = mybir.dt.bfloat16
F8 = mybir.dt.float8e4
W8SCALE = 16.0  # fp8 weight pre-scale (folded back out via the exp scale)
NDT = 4  # DoubleRow K-tiles over the embedding dim (4 x (128x2))
B, T, C = 2, 2048, 1024
H, D = 16, 64
SCALE = 1.0 / 32.0  # 1/sqrt(C)
S_SCALE = SCALE / (W8SCALE * W8SCALE)  # q,k carry a 16x pre-scale each
N_CORES = 8
NKT = C // 128  # 8 K-tiles over the embedding dim
NTK = T // 128  # 16 Tk tiles
NI = T // 512  # 4 Tq chunks
EXP = mybir.ActivationFunctionType.Exp

_nc_cache = {}


def build_kernel(repeats=1, hmix=False):
    key = (repeats, hmix)
    if key in _nc_cache:
        return _nc_cache[key]

    nc = bacc.Bacc("TRN2", target_bir_lowering=False, debug=False)

    # all DRAM inputs are host-packed so every DMA reads per-partition
    # CONTIGUOUS bytes: strided DRAM APs cost 1-4us of descriptor
    # generation on the issuing sequencer, which stalled the lead-in
    xc_d = nc.dram_tensor("xc", [128, NI, NKT, 512], BF, kind="ExternalInput").ap()
    xq8_d = nc.dram_tensor("xq8", [128, NI, NDT, 2, 512], F8, kind="ExternalInput").ap()
    wqk8_d = nc.dram_tensor("wqk8", [128, 2, NDT, 2, 256], F8, kind="ExternalInput").ap()
    wv_d = nc.dram_tensor("wv", [128, NKT, 256], BF, kind="ExternalInput").ap()
    wo_d = nc.dram_tensor("wo", [128, 2, C], BF, kind="ExternalInput").ap()
    y_d = nc.dram_tensor("y", [T, C], BF, kind="ExternalOutput").ap()

    # mask[p, i] = 1 iff i >= p : triangular strip at the causal boundary
    mask_np = (
        np.arange(128)[None, :] >= np.arange(128)[:, None]
    ).astype(ml_dtypes.bfloat16)
    mask_d = nc.inline_tensor(mask_np, "mask_tri").ap()

    with tile.TileContext(nc) as tc:
        with (
            tc.tile_pool(name="persist", bufs=1) as pp,
            tc.tile_pool(name="ppool", bufs=20) as ppool,
            tc.tile_pool(name="spool", bufs=4) as spool,
            tc.tile_pool(name="qkf", bufs=4) as qkf_pool,
            tc.tile_pool(name="ps_s", bufs=2, space="PSUM") as ps_s,
            tc.tile_pool(name="ps_o", bufs=2, space="PSUM") as ps_o,
            tc.tile_pool(name="ps_mm", bufs=2, space="PSUM") as ps_mm,
        ):
            # ---- critical-path DMAs first.  HWDGE issue is ~626ns serial
            # per DMA, so the pre-fold lead-in keeps the DMA count minimal:
            # scalar queue [wqk8, wv, folds...], sync queue [xq8[0], xc[0],
            # mask, rest] ----
            wqk8 = pp.tile([128, 2, NDT, 2, 256], F8, tag="wqk8")
            nc.scalar.dma_start(wqk8[:, 0, :, :, :], wqk8_d[:, 0, :, :, :])
            xq8 = [
                pp.tile([128, NDT, 2, 512], F8, tag=f"xq8{c}", name=f"xq8_{c}")
                for c in range(NI)
            ]
            nc.sync.dma_start(xq8[0][:, 0:2, :, :], xq8_d[:, 0, 0:2, :, :])
            nc.scalar.dma_start(wqk8[:, 1, :, :, :], wqk8_d[:, 1, :, :, :])
            nc.sync.dma_start(xq8[0][:, 2:4, :, :], xq8_d[:, 0, 2:4, :, :])
            wv_big = pp.tile([128, NKT, 256], BF, tag="wv")
            nc.scalar.dma_start(wv_big[:, :, :], wv_d[:, :, :])
            # x for the V projection; one contiguous DMA per chunk
            xc = [
                pp.tile([128, NKT, 512], BF, tag=f"xc{c}", name=f"xc{c}")
                for c in range(NI)
            ]
            nc.sync.dma_start(xc[0][:, 0:4, :], xc_d[:, 0, 0:4, :])
            nc.sync.dma_start(xc[0][:, 4:8, :], xc_d[:, 0, 4:8, :])
            mask = pp.tile([128, 128], BF, tag="mask")
            nc.sync.dma_start(mask[:], mask_d[:])
            # x for chunks >= 1 and wo are DMA'd lazily at the start of the
            # PREVIOUS chunk's attention: the DMA device serializes
            # transfers in issue order, and the chunk-c fold DMAs must not
            # queue behind 3 chunks of x traffic (2.9us each)
            wo_big = pp.tile([128, 2, C], BF, tag="wo")

            def emit_late_dmas(c):  # called at start of attention chunk c
                if R[0] > 0:
                    return  # inputs already resident on later repeats
                if c + 1 < NI:
                    nc.sync.dma_start(
                        xq8[c + 1][:, :, :, :], xq8_d[:, c + 1, :, :, :]
                    )
                    nc.sync.dma_start(xc[c + 1][:, :, :], xc_d[:, c + 1, :, :])
                if c == 1:
                    nc.sync.dma_start(wo_big[:, :, :], wo_d[:, :, :])

            # preload the exp table while DMAs stream (saves ~1.3us later)
            warm_in = pp.tile([1, 2], FP, tag="warm_in")
            warm_out = pp.tile([1, 2], FP, tag="warm_out")
            nc.vector.memset(warm_in[:], 0.0)
            nc.scalar.activation(warm_out[:], warm_in[:], EXP, scale=SCALE)

            # ---- persistent activations ----
            # qkf8[pair][c]: [128, 1024] f8 staging (q cols 0:512, k cols
            # 512:1024), rows in DR-plane order [e0:32|o0:32|e32:64|o32:64]
            # (host-permuted W cols).  Folded by 2 DMAs into qkdr[pair][c]
            # [64, 2, 1024]: partition p<32 head-even, 32:64 head-odd; ko
            # plane 0 = dims 0:32, plane 1 = dims 32:64.  S matmuls then run
            # fp8 DoubleRow (Ki=32, K=64) at half PE cost.
            qkf8 = [[None] * NI for _ in range(2)]
            qkdr = [
                [pp.tile([64, 2, 1024], F8, tag=f"qkd{p}_{i}", name=f"qkd{p}_{i}") for i in range(NI)]
                for p in range(2)
            ]
            otstc = [
                [pp.tile([128, 512], BF, tag=f"ot{p}_{i}", name=f"ot{p}_{i}") for i in range(NI)]
                for p in range(2)
            ]
            # vaug[t] = [Vh0|1|Vh1|1|Vh2|1|Vh3|1]; ones set once, V cols
            # rewritten per repeat by the strided copy from the V psum
            vaug = [pp.tile([128, 260], BF, tag=f"va{t}", name=f"va{t}") for t in range(NTK)]
            for t in range(NTK):
                nc.vector.memset(vaug[t][:, 64:260:65], 1.0)

            R = [0]

            # ---- PE filler machinery: queues of (cost_ns, emit_fn).
            # proj fillers must all land before the next attention chunk;
            # yproj fillers may linger until the final drain ----
            fillers_proj = []
            fillers_y = []
            debt = [0.0]
            y_reserve = [0]

            def pay_debt(max_pops=3, allow_proj=True):
                # cap pops per call so a debt spike cannot push the next
                # attention group's S matmuls out by more than ~one exp.
                # allow_proj=False while the next chunk's x transfers are
                # still in flight: a popped projection matmul would wait on
                # them IN the in-order PE stream, stalling attention behind
                while max_pops > 0 and debt[0] > 0.0 and (
                    (fillers_proj and allow_proj)
                    or len(fillers_y) > y_reserve[0]
                ):
                    q = (
                        fillers_proj
                        if (fillers_proj and allow_proj)
                        else fillers_y
                    )
                    cost, fn = q.pop(0)
                    fn()
                    debt[0] -= cost
                    max_pops -= 1

            def drain_proj_fillers():
                while fillers_proj:
                    _, fn = fillers_proj.pop(0)
                    fn()
                debt[0] = 0.0

            def drain_y_fillers(on_act=False):
                while fillers_y:
                    _, fn = fillers_y.pop(0)
                    fn(on_act=on_act)

            def drain_all_fillers():
                drain_proj_fillers()
                drain_y_fillers(on_act="tail")

            # ---- emission units ----
            def emit_qk_group(c, nm, pair, copy_on_act=False):
                qk = 0 if nm == "q" else 1
                ps = ps_mm.tile([128, 512], FP, tag="mm",
                                name=f"ps{nm}{c}_{pair}_r{R[0]}")
                for dt in range(NDT):
                    nc.tensor.matmul(
                        ps[:],
                        lhsT=wqk8[:, qk, dt, :, pair * 128 : pair * 128 + 128],
                        rhs=xq8[c][:, dt, :, :],
                        start=(dt == 0),
                        stop=(dt == NDT - 1),
                        perf_mode=mybir.MatmulPerfMode.DoubleRow,
                    )
                # this copy gates the next chunk's whole attention stream;
                # schedule it ahead of other queued DVE work
                if nm == "q":
                    qkf8[pair][c] = qkf_pool.tile(
                        [128, 1024], F8, tag="qkf", name=f"qkf{pair}_{c}_r{R[0]}"
                    )
                half = slice(0, 512) if nm == "q" else slice(512, 1024)
                with tc.high_priority(60):
                    if copy_on_act:
                        nc.scalar.copy(qkf8[pair][c][:, half], ps[:])
                    else:
                        nc.vector.tensor_scalar_mul(
                            qkf8[pair][c][:, half], ps[:], 1.0
                        )
                if nm == "k":  # both halves staged -> fold into DR layout.
                    # sync queue: a DMA issue seq-WAITS on its input dep
                    # while holding its queue, so folds must not share a
                    # queue with latency-critical issues (exp is on ACT)
                    with tc.high_priority(60):
                        nc.sync.dma_start(
                            qkdr[pair][c][:, 0, :], qkf8[pair][c][0:64, :]
                        )
                        nc.sync.dma_start(
                            qkdr[pair][c][:, 1, :], qkf8[pair][c][64:128, :]
                        )

            def emit_v_group(t):
                c, ts = t // 4, t % 4
                ps = ps_mm.tile([128, 512], FP, tag="mm",
                                name=f"psv{t}_r{R[0]}")
                for kk in range(NKT):
                    nc.tensor.matmul(
                        ps[:, 0:256],
                        lhsT=xc[c][:, kk, ts * 128 : ts * 128 + 128],
                        rhs=wv_big[:, kk, :],
                        start=(kk == 0),
                        stop=(kk == NKT - 1),
                    )
                with tc.high_priority(60):
                    nc.vector.tensor_copy(
                        vaug[t][:].rearrange("p (n d) -> p n d", n=4)[:, :, 0:64],
                        ps[:, 0:256].rearrange("p (n d) -> p n d", n=4),
                    )

            def emit_proj_chunk(c):
                # k-copies ride the (idle) ACT engine at the lead-in so the
                # fold does not wait behind the q-copy on DVE; V t1..t3 are
                # deferred into the attention stream so the first S/exp is
                # not queued behind 24 V matmuls on the in-order PE stream
                on_act = R[0] == 0
                for pair in range(2):
                    emit_qk_group(c, "q", pair)
                    emit_qk_group(c, "k", pair, copy_on_act=on_act)
                emit_v_group(4 * c)
                for t in range(4 * c + 1, 4 * c + 4):
                    fillers_proj.insert(
                        t - 4 * c - 1, (870, lambda t=t: emit_v_group(t))
                    )

            def proj_chunk_fillers(c):
                for pair in range(2):
                    fillers_proj.append((430, lambda pair=pair: emit_qk_group(
                        c, "q", pair)))
                    fillers_proj.append((430, lambda pair=pair: emit_qk_group(
                        c, "k", pair)))
                for t in range(4 * c, 4 * c + 4):
                    fillers_proj.append((870, lambda t=t: emit_v_group(t)))

            yt_chunk = [
                pp.tile([128, 4, 1024], BF, tag=f"yc{c}", name=f"yc{c}")
                for c in range(NI)
            ]
            y_done = {}

            def emit_yproj_unit(t, nch, on_act=False):
                ps = ps_mm.tile([128, 512], FP, tag="mm",
                                name=f"psy{t}_{nch}_r{R[0]}")
                for pair in range(2):
                    nc.tensor.matmul(
                        ps[:],
                        lhsT=otstc[pair][t // 4][
                            :, (t % 4) * 128 : (t % 4) * 128 + 128
                        ],
                        rhs=wo_big[:, pair, nch * 512 : nch * 512 + 512],
                        start=(pair == 0),
                        stop=(pair == 1),
                    )
                c = t // 4
                dst = yt_chunk[c][:, t % 4, nch * 512 : nch * 512 + 512]
                # mid-run copies go to DVE (ACT is exp-saturated); in the
                # reserve drain the DVE queue is full of normalize work so
                # use ACT; at the tail both are idle, so alternate
                if on_act == "act" or (
                    on_act == "tail" and (2 * t + nch) % 2 == 0
                ):
                    nc.scalar.copy(dst, ps[:])
                else:
                    nc.vector.tensor_copy(dst, ps[:])
                y_done[c] = y_done.get(c, 0) + 1
                if y_done[c] in (2, 4, 6, 8):  # quarter-chunk -> DMA it
                    lo = y_done[c] // 2 - 1
                    nc.sync.dma_start(
                        y_d[c * 512 + lo * 128 : c * 512 + lo * 128 + 128, :]
                        .rearrange("(n p) d -> p n d", p=128),
                        yt_chunk[c][:, lo : lo + 1, :],
                    )
                    if y_done[c] == 8:
                        y_done[c] = 0

            def emit_attention(I, last=False, pairs=(0, 1)):
                emit_attention_body(I, last, pairs)
                if 1 in pairs:
                    for t in range(4 * I, 4 * I + 4):
                        for nch in range(2):
                            fillers_y.append(
                                (430,
                                 lambda t=t, nch=nch, **kw: emit_yproj_unit(
                                     t, nch, **kw))
                            )

            def emit_attention_body(I, last, pairs):
                jmax = 4 * I + 4
                for pair in pairs:
                    oT = [
                        ps_o.tile([65, 512], FP, tag="oT",
                                  name=f"o{I}_{pair}_{h}_r{R[0]}")
                        for h in range(2)
                    ]

                    def emit_o(g, tiles):
                        j0 = 2 * g
                        for h in range(2):
                            p_sb = tiles[h]
                            head = 2 * pair + h
                            for dj in range(2):
                                j = j0 + dj
                                z = max(0, j * 128 - I * 512)
                                nc.tensor.matmul(
                                    oT[h][:, z:512],
                                    lhsT=vaug[j][:, 65 * head : 65 * head + 65],
                                    rhs=p_sb[:, dj * 512 + z : dj * 512 + 512],
                                    start=(j == 0),
                                    stop=(j == jmax - 1),
                                )

                    prev = None
                    for g in range(jmax // 2):
                        j0 = 2 * g
                        diag = j0 >= 4 * I
                        zs = [max(0, (j0 + dj) * 128 - I * 512) for dj in range(2)]
                        cur = []
                        act_ns = 0.0
                        cols = 0
                        for h in range(2):
                            hsl = slice(32 * h, 32 * h + 32)
                            s_ps = ps_s.tile([128, 1024], FP, tag="s",
                                             name=f"s{I}_{pair}_{h}_{g}_r{R[0]}")
                            for dj in range(2):
                                j = j0 + dj
                                z = zs[dj]
                                kof = 512 + (j % 4) * 128
                                nc.tensor.matmul(
                                    s_ps[:, dj * 512 + z : dj * 512 + 512],
                                    lhsT=qkdr[pair][j // 4][
                                        hsl, :, kof : kof + 128
                                    ],
                                    rhs=qkdr[pair][I][hsl, :, z:512],
                                    start=True,
                                    stop=True,
                                    perf_mode=mybir.MatmulPerfMode.DoubleRow,
                                )
                                cols += 512 - z
                            p_sb = ppool.tile([128, 1024], BF, tag="p",
                                              name=f"p{I}_{pair}_{h}_{g}_r{R[0]}")
                            if not diag or zs[0] == 0:
                                # diag group with z0=0: one act over the whole
                                # tile; the uncomputed gap [512:512+z1] holds
                                # stale S values, bounded so exp stays finite,
                                # and nothing downstream reads it
                                nc.scalar.activation(p_sb[:], s_ps[:], EXP,
                                                     scale=S_SCALE)
                                act_ns += 1024 * 0.833 + 185
                            else:
                                # one strided act at the smaller z: the
                                # over-covered [512+z0:512+z1] region holds
                                # bounded stale S (exp stays finite) and is
                                # never read downstream
                                nc.scalar.activation(
                                    p_sb[:].rearrange(
                                        "p (d t) -> p d t", d=2
                                    )[:, :, zs[0] : 512],
                                    s_ps[:].rearrange(
                                        "p (d t) -> p d t", d=2
                                    )[:, :, zs[0] : 512],
                                    EXP,
                                    scale=S_SCALE,
                                )
                                act_ns += 2 * (512 - zs[0]) * 0.833 + 185
                            for dj in range(2):
                                j = j0 + dj
                                if j >= 4 * I:
                                    z = zs[dj]
                                    ssl = slice(dj * 512 + z, dj * 512 + z + 128)
                                    nc.vector.tensor_mul(
                                        p_sb[:, ssl], p_sb[:, ssl], mask[:]
                                    )
                            cur.append(p_sb)
                        # ACT-vs-PE deficit for this group: exp time vs the
                        # S (fp8 DR, 0.2083/col) + O (bf16, 0.4167/col) time
                        debt[0] += act_ns - cols * 0.625
                        if prev is not None:
                            emit_o(g - 1, prev)
                        pay_debt()
                        prev = cur
                    emit_o(jmax // 2 - 1, prev)
                    if last and pair == 1:
                        # reserved y units keep PE warm through the final
                        # normalize chain
                        y_reserve[0] = 0
                        drain_y_fillers(on_act="act")
                    # normalize O^T by the PSUM row-64 denominator;
                    # h=1 first: its chain is longer (partition-shift DMA)
                    for h in (1, 0):
                        recip = spool.tile([1, 512], FP, tag="recip",
                                           name=f"rc{I}_{pair}_{h}_r{R[0]}")
                        nc.vector.reciprocal(recip[:], oT[h][64:65, :])
                        bcast = spool.tile([64, 512], FP, tag="bcast",
                                           name=f"bc{I}_{pair}_{h}_r{R[0]}")
                        nc.gpsimd.partition_broadcast(bcast[:], recip[:])
                        if h == 0:
                            nc.vector.tensor_mul(
                                otstc[pair][I][0:64, :], oT[h][0:64, :], bcast[:]
                            )
                        else:
                            onrm = spool.tile([64, 512], BF, tag="onrm",
                                              name=f"on{I}_{pair}_r{R[0]}")
                            nc.vector.tensor_mul(onrm[:], oT[h][0:64, :], bcast[:])
                            # partition shift 0->64 needs a DMA
                            nc.sync.dma_start(otstc[pair][I][64:128, :], onrm[:])


            # ---- main emission.  Attention instructions carry high
            # scheduler priority (they form the serial latency chain:
            # S -> exp -> mask -> O -> normalize); projections and
            # y-projection units are normal priority, so the greedy
            # scheduler slots them into PE whenever attention work is
            # not ready ----
            for rep in range(repeats):
                R[0] = rep
                emit_proj_chunk(0)
                for c in range(NI):
                    emit_late_dmas(c)
                    if c + 1 < NI:
                        proj_chunk_fillers(c + 1)
                    else:
                        y_reserve[0] = 8
                    if c < 2:
                        # early chunks have little attention work: drain
                        # next-chunk projections eagerly so their folds land
                        # before this chunk's exp stream runs dry
                        debt[0] += 3500.0 if c == 0 else 1000.0
                    emit_attention(c, last=(c == NI - 1))
                    drain_proj_fillers()
                y_reserve[0] = 0
                drain_all_fillers()

    nc.compile()
    _nc_cache[key] = nc
    return nc


def make_in_maps(x, Wq, Wk, Wv, Wo):
    x = np.asarray(x, dtype=np.float32)
    Wq = np.asarray(Wq, dtype=np.float32)
    Wk = np.asarray(Wk, dtype=np.float32)
    Wv = np.asarray(Wv, dtype=np.float32)
    Wo = np.asarray(Wo, dtype=np.float32)
    bf = ml_dtypes.bfloat16
    f8 = ml_dtypes.float8_e4m3fn

    def dr_pack(a):  # [C, m] -> [128, NDT, 2, m] with k = 256*dt + ki + 128*ko
        return np.ascontiguousarray(
            a.reshape(4, 2, 128, -1).transpose(2, 0, 1, 3)
        )

    # permute the M (output-row) dim of the q/k projection weights so the
    # PSUM rows land in DR-plane order per pair: [e0:32|o0:32|e32:64|o32:64]
    # (e = even head dims, o = odd head dims of the pair)
    qk_perm = np.concatenate(
        [b * 128 + np.r_[0:32, 64:96, 32:64, 96:128] for b in range(2)]
    )

    in_maps = []
    for c in range(N_CORES):
        b, hg = c // 4, c % 4
        sl = slice(256 * hg, 256 * hg + 256)
        xTb = x[b].T  # [C, T]
        xq8 = dr_pack(xTb.astype(f8))  # [128, 4, 2, T]
        in_maps.append(
            {
                # per-partition-contiguous packings (cheap DMA descriptors)
                "xc": np.ascontiguousarray(
                    xTb.astype(bf).reshape(8, 128, 4, 512).transpose(1, 2, 0, 3)
                ),
                "xq8": np.ascontiguousarray(
                    xq8.reshape(128, 4, 2, 4, 512).transpose(0, 3, 1, 2, 4)
                ),
                "wqk8": np.ascontiguousarray(np.stack([
                    dr_pack((Wq[sl, :].T * W8SCALE).astype(f8)[:, qk_perm]),
                    dr_pack((Wk[sl, :].T * W8SCALE).astype(f8)[:, qk_perm]),
                ], axis=1)),
                "wv": np.ascontiguousarray(
                    Wv[sl, :].T.astype(bf).reshape(8, 128, 256).transpose(1, 0, 2)
                ),
                "wo": np.ascontiguousarray(
                    Wo[:, sl].T.astype(bf).reshape(2, 128, 1024).transpose(1, 0, 2)
                ),
            }
        )
    return in_maps


def run_spmd(in_maps, trace=False, repeats=1, **kw):
    nc = build_kernel(repeats)
    return run_bass_kernel_spmd(nc, in_maps, list(range(N_CORES)), trace=trace, **kw)


def gather(results, bo):
    bo = np.asarray(bo, dtype=np.float32)
    y = np.empty((B, T, C), dtype=np.float32)
    for b in range(B):
        acc = results[4 * b]["y"].astype(np.float32).copy()
        for g in range(1, 4):
            acc += results[4 * b + g]["y"].astype(np.float32)
        y[b] = acc + bo[None, :]
    return y


def kernel(x, Wq, Wk, Wv, Wo, bo):
    res = run_spmd(make_in_maps(x, Wq, Wk, Wv, Wo))
    return gather(res.results, bo)

